# revision 4
# baseline (speedup 1.0000x reference)
"""Trainium2 Bass kernel v2 for nn_Attention (B=4, N=2048, C=768, H=12).

Sharding: 8 cores = 4 batches x 2 head-groups (6 heads each). Each core
computes qkv for its 6 heads, attention, and the full projection partial
(all 3 head-pairs summed on-chip); the host adds the 2 group partials + bias.

vs v1:
- exp() emits fp8e4 (e4m3) probabilities directly, with a global logit shift
  C (softmax-invariant) keeping exp in e4m3's dynamic range.
- PV runs as fp8 DoubleRow matmuls (2 j-chunk k-tiles per instruction, 0.5
  cycles/row) with v = v_hi + v_lo error-feedback pair; a ones column in
  v_hi's 65th row gives Z for free. PE time for PV halves vs the fp32r M=65
  scheme.
- The projection contracts all 3 pairs on-chip (bf16) into one bf16 output;
  the host sums 2 partials instead of 6.
- exp is the span-setting engine (~205us ACT); the schedule keeps the ACT
  stream gapless: minimal lead-in (coarse single-DMA weight/x loads, x issued
  from the otherwise-idle ACT queue), PV interleaved with one jj lag so the
  S->exp chain never queues behind PV, and proj lagged one block.
"""

import os
import sys
from contextlib import ExitStack

if "/opt/trn_rl_repo" not in sys.path:
    sys.path.insert(0, "/opt/trn_rl_repo")

import numpy as np
import ml_dtypes

import concourse.bass as bass
import concourse.mybir as mybir
import concourse.tile as tile
from concourse import bass_utils

F32 = mybir.dt.float32
BF16 = mybir.dt.bfloat16
F8 = mybir.dt.float8e4
U8 = mybir.dt.uint8
DRMODE = mybir.MatmulPerfMode.DoubleRow

B, N, C = 4, 2048, 768
NH, D = 12, 64
SCALE = D ** -0.5
HPC = NH // 2          # heads per core
F = HPC * D            # 384 per-core features per projection
P = 128
CO = C // P            # 6 contraction chunks
FO = F // P            # 3 head pairs
NO = N // P            # 16 token chunks
NJJ = NO // 2          # 8 j-chunk pairs (DoubleRow k-tiles)
NCORES = 8
VSTRIDE = 80           # v8hi per-head stride (must be %16==0 for DR lhsT)

C_SHIFT = float(os.environ.get("KERNEL_C_SHIFT", "4.0"))
A_SCH = 8.0 / np.log(2.0)
B_SCH = 55.5
# j-chunk indices (0..15) whose exp runs on the DVE via corrected Schraudolph.
# Default empty: the bit-trick exp fails the 2e-2 gate even at 25% mixing.
_dve_js = os.environ.get("KERNEL_DVE_JS", "")
DVE_JS = set(int(x) for x in _dve_js.split(",") if x != "")

MM_DT = mybir.dt.float32r


def _d(ap):
    return ap.bitcast(MM_DT)


def _r(ap):
    """Cast a producer OUT AP feeding an fp32r matmul (rounding chain)."""
    return ap.bitcast(MM_DT)


def _split_multiwaits(nc):
    """Walrus accepts at most ONE sync-wait per instruction: split extras
    into single-wait NOPs queued just before (FIFO-equivalent)."""
    ctr = 0
    for f in nc.m.functions:
        for blk in f.blocks:
            insts = blk.instructions
            out = []
            changed = False
            for ins in insts:
                si = ins.sync_info
                if si is not None and len(si.on_wait) > 1:
                    changed = True
                    waits = list(si.on_wait)
                    for ww in waits[:-1]:
                        nop = mybir.InstNoOp(name=f"zzsplitw_{ctr}", ins=[], outs=[])
                        ctr += 1
                        nop.engine = ins.engine
                        nop.sync_info = mybir.SyncInfo(on_wait=[ww], on_update=[])
                        out.append(nop)
                    ins.sync_info = mybir.SyncInfo(
                        on_wait=[waits[-1]], on_update=list(si.on_update)
                    )
                out.append(ins)
            if changed:
                blk.instructions = out
    return nc


def _emit(nc, tc, ctx):
    # x n4-major: [n4][128, CO, 512] bf16 so one cheap DMA per 512-token slice
    xTn = nc.dram_tensor("xTn", [4, P, CO, 512], BF16, kind="ExternalInput").ap()
    # five contiguous weight sections [C, w]; loaded with ONE rearranged DMA
    # each into [128, CO, w] (HWDGE descgen is a serial device: fewer DMAs)
    SECS = ((0, P), (F, F + P), (2 * F, 3 * F), (P, F), (F + P, 2 * F))
    wq_secs = {
        lo: nc.dram_tensor(f"wq{lo}", [C, hi - lo], BF16, kind="ExternalInput").ap()
        for lo, hi in SECS
    }
    wprojT = nc.dram_tensor("wprojT", [FO, P, C], BF16, kind="ExternalInput").ap()
    out3 = nc.dram_tensor("out3", [N, C], BF16, kind="ExternalOutput").ap()

    persist = ctx.enter_context(tc.tile_pool(name="persist", bufs=1))

    ones_pe = persist.tile([P, 64], F32, tag="ones_pe")
    warm_in = persist.tile([64, 512], F32, tag="warm_in")
    nc.gpsimd.memset(warm_in, 0.0)
    nc.vector.memset(ones_pe, 1.0)

    # q/k [feature, token] fp32, per (pair, 512-token chunk)
    q_sb = [[persist.tile([P, 512], F32, tag=f"q{fo}_{n4}", name=f"q{fo}_{n4}")
             for n4 in range(4)] for fo in range(FO)]
    k_sb = [[persist.tile([P, 512], F32, tag=f"k{fo}_{n4}", name=f"k{fo}_{n4}")
             for n4 in range(4)] for fo in range(FO)]
    # v fp8 hi/lo per jj: [j-in-chunk, kt, head-strided columns]
    v8hi = [persist.tile([P, 2, HPC * VSTRIDE], F8, tag=f"v8hi{jj}", name=f"v8hi{jj}")
            for jj in range(NJJ)]
    v8lo = [persist.tile([P, 2, HPC * D], F8, tag=f"v8lo{jj}", name=f"v8lo{jj}")
            for jj in range(NJJ)]
    # attention output (normalized) per pair, [feature, token] bf16
    ot_sb = [persist.tile([P, N], BF16, tag=f"ot{pr}", name=f"ot{pr}")
             for pr in range(FO)]
    wp_sb = persist.tile([P, FO, C], BF16, tag="wp")
    # pair-2 odd-head rows of wp copied to partitions 0:64 (tail split-K proj
    # reads the normalized odd half from nt, which lives at partitions 0:64)
    wp2dn = persist.tile([64, C], BF16, tag="wp2dn")

    # ones row at partition 64 for the tail's PE-broadcast of 1/Z (memset
    # first: the PE warm-up dummies depend on it)
    ones_pe0 = None
    # ones columns of v8hi (Z rows)
    for jj in range(NJJ):
        nc.gpsimd.memset(
            v8hi[jj].rearrange("p t (h s) -> p t h s", s=VSTRIDE)[:, :, :, D : D + 1],
            1.0,
        )
    # bias AP for exp's global logit shift (softmax-invariant)
    bias_c = persist.tile([P, 1], F32, tag="biasc")
    nc.vector.memset(bias_c, -C_SHIFT)
    # dummy exp: pull the ACT table load into the DMA lead-in window
    expwarm = persist.tile([P, 4], F32, tag="expwarm")
    nc.vector.memset(expwarm, 0.0)
    nc.scalar.activation(
        out=expwarm, in_=expwarm, func=mybir.ActivationFunctionType.Exp, scale=1.0,
        bias=bias_c,
    )

    with (
        tc.tile_pool(name="wqp", bufs=1) as wqp,
        tc.tile_pool(name="xs", bufs=4) as xs_pool,
        tc.tile_pool(name="p8p", bufs=12) as p8_pool,
        tc.tile_pool(name="rp", bufs=2) as r_pool,
        tc.tile_pool(name="outp", bufs=2) as outp,
        tc.tile_pool(name="rd", bufs=3, space="DRAM") as rd_pool,
        tc.tile_pool(name="ps_st", bufs=2, space="PSUM") as ps_st,
        tc.tile_pool(name="ps_po", bufs=2, space="PSUM") as ps_po,
        tc.tile_pool(name="ps_mm", bufs=2, space="PSUM") as ps_mm,
    ):
        wq_tiles = {}

        def load_wq(lo, hi):
            w = hi - lo
            t = wqp.tile([P, CO, w], BF16, tag=f"wq_{lo}", name=f"wq_{lo}")
            wq_tiles[lo] = t
            nc.sync.dma_start(
                out=t,
                in_=wq_secs[lo].rearrange("(co p) w -> p co w", p=P),
            )

        def wq_slice(foff, co, width=P):
            for lo, hi in SECS:
                if lo <= foff and foff + width <= hi:
                    return wq_tiles[lo][:, co, foff - lo : foff - lo + width]
            raise KeyError(foff)

        # ---- QKV -------------------------------------------------------
        def emit_qk_group(fo, n4, which, xt4):
            """One q or k psum group [128, 512] -> q_sb/k_sb fp32 (bf16 mm)."""
            dst = (q_sb if which == "q" else k_sb)[fo][n4]
            foff = fo * P if which == "q" else F + fo * P
            pq = ps_mm.tile([P, 512], F32, tag="mm", name=f"p{which}{fo}_{n4}")
            for co in range(CO):
                nc.tensor.matmul(
                    pq, wq_slice(foff, co), xt4[:, co, :],
                    start=(co == 0), stop=(co == CO - 1),
                )
            nc.vector.tensor_copy(out=_r(dst), in_=pq)

        def emit_x_dma(n4):
            xt4 = xs_pool.tile([P, CO, 512], BF16, tag="xt4", name=f"xt4_{n4}")
            nc.sync.dma_start(out=xt4, in_=xTn[n4, :, :, :])
            return xt4

        def emit_v_chunk(no, xtv):
            """v for one 128-token chunk -> v8hi/v8lo fp8 (+ones col in hi)."""
            pv = ps_mm.tile([P, F], F32, tag="mm", name=f"pv_{no}")
            for co in range(CO):
                nc.tensor.matmul(
                    pv,
                    xtv[:, co, (no % 4) * P : (no % 4 + 1) * P],
                    wq_slice(2 * F, co, F),
                    start=(co == 0), stop=(co == CO - 1),
                )
            jj, half = divmod(no, 2)
            hi = v8hi[jj].rearrange("p t (h s) -> p t h s", s=VSTRIDE)
            nc.vector.tensor_copy(
                out=hi[:, half, :, 0:D],
                in_=pv.rearrange("p (h d) -> p h d", h=HPC),
            )
            lo = v8lo[jj].rearrange("p t (h d) -> p t h d", d=D)
            nc.vector.tensor_sub(
                out=lo[:, half, :, :],
                in0=pv.rearrange("p (h d) -> p h d", h=HPC),
                in1=hi[:, half, :, 0:D],
            )

        # ---- attention -------------------------------------------------
        def emit_pv(pr, jj, p8, po_a, po_b):
            hA, hB = 2 * pr, 2 * pr + 1
            vh = v8hi[jj]
            vl = v8lo[jj]
            for h, po, icol in ((hA, po_a, 0), (hB, po_b, 512)):
                nc.tensor.matmul(
                    po,
                    vh[:, :, h * VSTRIDE : h * VSTRIDE + 65],
                    p8[:, :, icol : icol + 512],
                    start=(jj == 0), stop=False,
                    perf_mode=DRMODE,
                    skip_group_check=True,
                )
                nc.tensor.matmul(
                    po[0:64, :],
                    vl[:, :, h * D : h * D + 64],
                    p8[:, :, icol : icol + 512],
                    start=False, stop=(jj == NJJ - 1),
                    perf_mode=DRMODE,
                    skip_group_check=True,
                )

        def make_norm(pr, i4, po_a, po_b, pe_bcast=False):
            """Closure emitting the normalize chain for block (pr, i4).

            pe_bcast: broadcast 1/Z across partitions with a K=1 ones-matmul
            into a free PSUM slot instead of the DRAM bounce (tail only --
            saves the DMA round trip and keeps the PE warm for proj).
            """
            i0 = i4 * 512

            def norm_pe():
                rv = r_pool.tile([65, 1024], F32, tag="rv", name=f"rv_{pr}_{i4}")
                with nc.allow_low_precision(reason="1/Z feeds fp32r bcast matmul"):
                    nc.vector.reciprocal(out=_r(rv[64:65, 0:512]), in_=po_a[64:65, :])
                    nc.vector.reciprocal(out=_r(rv[64:65, 512:1024]), in_=po_b[64:65, :])
                rb_ps = ps_st.tile([64, 1024], F32, tag="st", name=f"rbps_{pr}_{i4}")
                for h in range(2):
                    nc.tensor.matmul(
                        rb_ps[:, h * 512 : h * 512 + 512],
                        _d(ones_pe[64:65, :]),
                        _d(rv[64:65, h * 512 : h * 512 + 512]),
                        start=True, stop=True,
                        tile_position=(64, 0),
                    )
                rb_sb = r_pool.tile([64, 2, 512], F32, tag="rb", name=f"rbs_{pr}_{i4}")
                nc.vector.tensor_copy(out=rb_sb, in_=rb_ps.rearrange("p (t n) -> p t n", t=2))
                nc.vector.tensor_mul(
                    out=ot_sb[pr][0:64, i0 : i0 + 512], in0=po_a[0:64, :],
                    in1=rb_sb[:, 0, :],
                )
                nt = r_pool.tile([64, 512], BF16, tag="nt", name=f"nt_{pr}_{i4}")
                nc.vector.tensor_mul(out=nt, in0=po_b[0:64, :], in1=rb_sb[:, 1, :])
                nc.sync.dma_start(out=ot_sb[pr][64:128, i0 : i0 + 512], in_=nt)
                tail_nt["nt"] = nt

            def norm():
                if pe_bcast:
                    return norm_pe()
                rv = r_pool.tile([65, 1024], F32, tag="rv", name=f"rv_{pr}_{i4}")
                nc.vector.reciprocal(out=rv[64:65, 0:512], in_=po_a[64:65, :])
                nc.vector.reciprocal(out=rv[64:65, 512:1024], in_=po_b[64:65, :])
                rdram = rd_pool.tile([1, 1024], F32, tag="rd", name=f"rd_{pr}_{i4}")
                nc.sync.dma_start(out=rdram, in_=rv[64:65, :])
                rb = r_pool.tile([64, 2, 512], F32, tag="rb", name=f"rb_{pr}_{i4}")
                nc.sync.dma_start(out=rb[:, 0, :], in_=rdram[0:1, 0:512].to_broadcast([64, 512]))
                nc.sync.dma_start(out=rb[:, 1, :], in_=rdram[0:1, 512:1024].to_broadcast([64, 512]))
                # multiply straight out of PSUM (DVE reads psum; po slot is
                # not needed again until ~a block later)
                nc.vector.tensor_mul(
                    out=ot_sb[pr][0:64, i0 : i0 + 512], in0=po_a[0:64, :], in1=rb[:, 0, :],
                )
                nt = r_pool.tile([64, 512], BF16, tag="nt", name=f"nt_{pr}_{i4}")
                nc.vector.tensor_mul(out=nt, in0=po_b[0:64, :], in1=rb[:, 1, :])
                nc.sync.dma_start(out=ot_sb[pr][64:128, i0 : i0 + 512], in_=nt)

            return norm

        def emit_attention(pr, interleave=None, cascade=False, carry_in=None):
            """Attention for pair pr over 4 i-blocks of 512.

            cascade=False: PV(jj) inline with a one-jj lag (j = 2jj+3), last
            PV + normalize at block end (pairs 1, 2 -- PE fits in ACT slack).
            cascade=True: block k's 16 PV instructions + its normalize run
            spread inside block k+1's j-loop (pair 0 -- makes room for the v
            projection inside blocks 0-1 without stalling the exp stream).
            The last block's closures are returned as carry for the next
            pair's first block.
            """
            pending = list(carry_in or [])

            def pop_pending():
                if pending:
                    pending.pop(0)()

            for i4 in range(4):
                po_a = ps_po.tile([65, 512], F32, tag="po", name=f"poA_{pr}_{i4}")
                po_b = ps_po.tile([65, 512], F32, tag="po", name=f"poB_{pr}_{i4}")
                p8s = []
                for j in range(NO):
                    kt = k_sb[pr][j // 4]
                    jo = (j % 4) * P
                    qt = q_sb[pr][i4]
                    stm = ps_st.tile([P, 1024], F32, tag="st", name=f"st_{pr}_{i4}_{j}")
                    nc.tensor.matmul(
                        stm[:, 0:512], _d(kt[0:64, jo : jo + P]), _d(qt[0:64, :]),
                        start=True, stop=True,
                    )
                    nc.tensor.matmul(
                        stm[:, 512:1024], _d(kt[64:128, jo : jo + P]), _d(qt[64:128, :]),
                        start=True, stop=True,
                    )
                    if j % 2 == 0:
                        p8 = p8_pool.tile([P, 2, 1024], F8, tag="p8",
                                          name=f"p8_{pr}_{i4}_{j // 2}")
                        p8s.append(p8)
                    p8 = p8s[j // 2]
                    if j in DVE_JS:
                        nc.vector.tensor_scalar(
                            out=p8[:, j % 2, :].bitcast(U8),
                            in0=stm,
                            scalar1=SCALE * A_SCH,
                            scalar2=B_SCH - C_SHIFT * A_SCH,
                            op0=mybir.AluOpType.mult,
                            op1=mybir.AluOpType.add,
                        )
                    else:
                        nc.scalar.activation(
                            out=p8[:, j % 2, :],
                            in_=stm,
                            func=mybir.ActivationFunctionType.Exp,
                            scale=SCALE,
                            bias=bias_c,
                        )
                    if interleave is not None:
                        interleave(i4, j)
                    if cascade:
                        # one pending item per odd j: PV(jj) lands at j=2jj+1,
                        # safely after v(2jj+1)'s write at even j
                        if j % 2 == 1:
                            pop_pending()
                    elif j >= 5 and j % 2 == 1:
                        emit_pv(pr, (j - 5) // 2, p8s[(j - 5) // 2], po_a, po_b)
                if cascade:
                    while pending:
                        pop_pending()
                    for jj in range(NJJ):
                        _jj, _p8 = jj, p8s[jj]
                        pending.append(
                            lambda _jj=_jj, _p8=_p8, _pa=po_a, _pb=po_b: emit_pv(
                                pr, _jj, _p8, _pa, _pb
                            )
                        )
                    pending.append(make_norm(pr, i4, po_a, po_b))
                else:
                    emit_pv(pr, NJJ - 2, p8s[NJJ - 2], po_a, po_b)
                    emit_pv(pr, NJJ - 1, p8s[NJJ - 1], po_a, po_b)
                    make_norm(pr, i4, po_a, po_b,
                              pe_bcast=(pr == 2 and i4 == 3))()
            return pending

        # ---- projection (all 3 pairs on-chip, bf16) --------------------
        proj_state = {"o_sb": None}
        tail_nt = {}

        def emit_proj(no_range, evac_act=False):
            for no in no_range:
                if no % 2 == 0:
                    proj_state["o_sb"] = outp.tile(
                        [P, 2, C], BF16, tag="o", name=f"o_{no}"
                    )
                o_sb = proj_state["o_sb"]
                for half in range(2):
                    pp = ps_mm.tile([P, 384], F32, tag="mm", name=f"pp_{no}_{half}")
                    for fo in range(FO):
                        nc.tensor.matmul(
                            pp,
                            ot_sb[fo][:, no * P : (no + 1) * P],
                            wp_sb[:, fo, half * 384 : half * 384 + 384],
                            start=(fo == 0), stop=(fo == FO - 1),
                        )
                    dst = o_sb[:, no % 2, half * 384 : half * 384 + 384]
                    if evac_act and half == 0:
                        nc.scalar.copy(out=dst, in_=pp)
                    else:
                        nc.vector.tensor_copy(out=dst, in_=pp)
                if no % 2 == 1:
                    nc.sync.dma_start(
                        out=out3[(no - 1) * P : (no + 1) * P, :].rearrange(
                            "(t p) c -> p t c", t=2
                        ),
                        in_=o_sb,
                    )

        # ---- schedule --------------------------------------------------
        # lead-in (bf16 DMAs pipeline with the k-group chain): wqk, x0, wq0,
        # x1, wv, x2, x3 -- k(0,n4) lands just before the exp stream needs it
        xt4_0 = xs_pool.tile([P, CO, 512], BF16, tag="xt4", name="xt4_0")
        nc.sync.dma_start(out=xt4_0[:, 0, :], in_=xTn[0, :, 0, :])
        nc.sync.dma_start(out=xt4_0[:, 1, :], in_=xTn[0, :, 1, :])
        load_wq(F, F + P)                               # k0 weights
        nc.sync.dma_start(out=xt4_0[:, 2, :], in_=xTn[0, :, 2, :])
        load_wq(0, P)                                   # q0 weights
        for co in range(3, CO):
            nc.sync.dma_start(out=xt4_0[:, co, :], in_=xTn[0, :, co, :])
        xt4s0 = [xt4_0]
        # warm the PE clock while the lead-in DMAs stream (HAM ramp)
        warm_sc = ps_st.tile([64, 1024], F32, tag="st", name="warm_sc")
        for _ in range(10):
            nc.tensor.matmul(
                warm_sc[:, 0:128],
                _d(warm_in[:, 0:64]),
                _d(warm_in[:, 0:128]),
                start=True, stop=True,
            )
        xt4s0.append(emit_x_dma(1))
        load_wq(2 * F, 3 * F)                           # v weights
        xt4s0.append(emit_x_dma(2))
        xt4s0.append(emit_x_dma(3))
        # fused k(0,0)+q(0,0): co-matmuls pipelined against x0 chunk arrivals
        pk0 = ps_mm.tile([P, 512], F32, tag="mm", name="pk00")
        pq0 = ps_mm.tile([P, 512], F32, tag="mm", name="pq00")
        for co in range(CO):
            nc.tensor.matmul(
                pk0, wq_slice(F, co), xt4_0[:, co, :],
                start=(co == 0), stop=(co == CO - 1),
            )
            nc.tensor.matmul(
                pq0, wq_slice(0, co), xt4_0[:, co, :],
                start=(co == 0), stop=(co == CO - 1),
            )
        nc.vector.tensor_copy(out=_r(k_sb[0][0]), in_=pk0)
        nc.vector.tensor_copy(out=_r(q_sb[0][0]), in_=pq0)
        nc.sync.dma_start(
            out=wp_sb, in_=wprojT.rearrange("fo p c -> p fo c"),
        )
        nc.sync.dma_start(out=wp2dn, in_=wprojT[2, 64:128, :])
        load_wq(P, F)                                   # q1/q2
        load_wq(F + P, 2 * F)                           # k1/k2

        # pair 0 (cascade PV): v chunks spread over blocks 0-1, q(0,i4+1)
        # at j==14, pair-1 qkv groups in blocks 2-3
        x_cache = {}
        qk1_slots = [(2, 2), (2, 6), (2, 10), (3, 2), (3, 4), (3, 8), (3, 10), (3, 12)]
        qk2_slots = [(0, 4), (0, 10), (1, 4), (1, 10), (2, 4), (2, 10), (3, 4), (3, 10)]
        qkn4 = [0, 1, 2, 3, 0, 1, 2, 3]

        def emit_qk_pair_group(pr, idx):
            n4 = qkn4[idx]
            if (pr, n4) not in x_cache:
                if pr == 2:
                    x_cache[(pr, n4)] = x_cache[(1, n4)]
                else:
                    x_cache[(pr, n4)] = emit_x_dma(n4)
            xt4 = x_cache[(pr, n4)]
            emit_qk_group(pr, n4, "q" if idx < 4 else "k", xt4)

        def inter0(i4, j):
            # k(0,1..3) pipelined against the x1..x3 DMA arrivals
            if i4 == 0 and j in (1, 5, 9):
                emit_qk_group(0, 1 + (j - 1) // 4, "k", xt4s0[1 + (j - 1) // 4])
            # v(no): 10 chunks late in block 0 (after wv lands), 6 in block 1
            if i4 == 0 and 6 <= j:
                emit_v_chunk(j - 6, xt4s0[(j - 6) // 4])
            elif i4 == 1 and j % 2 == 0 and 2 <= j <= 12:
                emit_v_chunk(10 + (j - 2) // 2, xt4s0[(10 + (j - 2) // 2) // 4])
            if j == 14 and i4 < 3:
                emit_qk_group(0, i4 + 1, "q", xt4s0[i4 + 1])
            if (i4, j) in qk1_slots:
                emit_qk_pair_group(1, qk1_slots.index((i4, j)))

        carry = emit_attention(0, interleave=inter0, cascade=True)

        def inter1(i4, j):
            if (i4, j) in qk2_slots:
                emit_qk_pair_group(2, qk2_slots.index((i4, j)))
            # drain pair-0 block-3's carried PVs + norm in block 0
            if i4 == 0 and j % 2 == 1 and carry:
                carry.pop(0)()
                if j == 13:
                    while carry:
                        carry.pop(0)()

        emit_attention(1, interleave=inter1)

        # attention 2: proj for block i4-1's chunks lands inside block i4
        def inter2(i4, j):
            if i4 >= 1 and j in (5, 8, 11, 14):
                no = 4 * (i4 - 1) + (5, 8, 11, 14).index(j)
                emit_proj([no])

        emit_attention(2, interleave=inter2)
        for _ in range(10):
            nc.tensor.matmul(
                warm_sc[:, 0:64], _d(ones_pe[0:64, :]), _d(ones_pe[0:64, :]),
                start=True, stop=True,
            )
        # tail proj: split pair-2's contraction so the odd-head half reads nt
        # (SBUF partitions 0:64) instead of waiting for the ot DMA-up
        nt3 = tail_nt["nt"]
        for no in range(12, 16):
            if no % 2 == 0:
                proj_state["o_sb"] = outp.tile([P, 2, C], BF16, tag="o", name=f"o_{no}")
            o_sb = proj_state["o_sb"]
            for half in range(2):
                pp = ps_mm.tile([P, 384], F32, tag="mm", name=f"pp_{no}_{half}")
                hc = slice(half * 384, half * 384 + 384)
                for fo in range(2):
                    nc.tensor.matmul(
                        pp, ot_sb[fo][:, no * P : (no + 1) * P], wp_sb[:, fo, hc],
                        start=(fo == 0), stop=False, skip_group_check=True,
                    )
                nc.tensor.matmul(
                    pp, ot_sb[2][0:64, no * P : (no + 1) * P], wp_sb[0:64, 2, hc],
                    start=False, stop=False, skip_group_check=True,
                )
                nc.tensor.matmul(
                    pp, nt3[:, (no % 4) * P : (no % 4 + 1) * P], wp2dn[:, hc],
                    start=False, stop=True, skip_group_check=True,
                )
                dst = o_sb[:, no % 2, hc]
                if half == 0:
                    nc.scalar.copy(out=dst, in_=pp)
                else:
                    nc.vector.tensor_copy(out=dst, in_=pp)
            if no >= 14:
                nc.sync.dma_start(
                    out=out3[no * P : (no + 1) * P, :],
                    in_=proj_state["o_sb"][:, no % 2, :],
                )
            elif no % 2 == 1:
                nc.sync.dma_start(
                    out=out3[(no - 1) * P : (no + 1) * P, :].rearrange(
                        "(t p) c -> p t c", t=2
                    ),
                    in_=proj_state["o_sb"],
                )


_NC_CACHE = {}


def build_bass():
    key = (C_SHIFT, tuple(sorted(DVE_JS)))
    if key in _NC_CACHE:
        return _NC_CACHE[key]
    nc = bass.Bass("TRN2")
    with tile.TileContext(nc) as tc:
        with ExitStack() as ctx:
            _emit(nc, tc, ctx)
    _split_multiwaits(nc)
    _NC_CACHE[key] = nc
    return nc


def make_in_maps(x, w_qkv, w_proj):
    x = np.asarray(x, dtype=np.float32)
    w_qkv = np.asarray(w_qkv, dtype=np.float32)
    w_proj = np.asarray(w_proj, dtype=np.float32)
    wq, wk, wv = w_qkv[0:C], w_qkv[C : 2 * C], w_qkv[2 * C : 3 * C]
    in_maps = []
    for c in range(NCORES):
        b, g = divmod(c, 2)
        sl = slice(g * F, (g + 1) * F)
        wslice = np.concatenate([wq[sl], wk[sl], wv[sl]], axis=0)  # [1152, 768]
        wT = np.ascontiguousarray(wslice.T)  # [768, 1152]
        xT = x[b].T  # [768, 2048]
        # [n4][128, co, 512] bf16
        xTn = np.ascontiguousarray(
            xT.reshape(CO, P, 4, 512).transpose(2, 1, 0, 3).astype(ml_dtypes.bfloat16)
        )
        wpT = np.ascontiguousarray(w_proj[:, sl].T)  # [384, 768]
        m = {
            "xTn": xTn,
            "wprojT": np.ascontiguousarray(
                wpT.reshape(FO, P, C).astype(ml_dtypes.bfloat16)
            ),
        }
        for lo, hi in ((0, 128), (384, 512), (768, 1152), (128, 384), (512, 768)):
            m[f"wq{lo}"] = np.ascontiguousarray(wT[:, lo:hi].astype(ml_dtypes.bfloat16))
        in_maps.append(m)
    return in_maps


def gather_output(parts, b_proj):
    outv = np.empty((B, N, C), np.float32)
    for b in range(B):
        outv[b] = parts[2 * b].astype(np.float32) + parts[2 * b + 1].astype(np.float32)
    outv += np.asarray(b_proj, dtype=np.float32)[None, None, :]
    return outv


def kernel(x, w_qkv, w_proj, b_proj, _run_kwargs=None):
    nc = build_bass()
    in_maps = make_in_maps(x, w_qkv, w_proj)
    res = bass_utils.run_bass_kernel_spmd(
        nc, in_maps, core_ids=list(range(NCORES)), **(_run_kwargs or {})
    )
    parts = [r["out3"] for r in res.results]
    outv = gather_output(parts, b_proj)
    if _run_kwargs is not None:
        kernel.last_results = res
    return outv


# revision 5
# speedup vs baseline: 1.0015x; 1.0015x over previous
"""Trainium2 Bass kernel v2 for nn_Attention (B=4, N=2048, C=768, H=12).

Sharding: 8 cores = 4 batches x 2 head-groups (6 heads each). Each core
computes qkv for its 6 heads, attention, and the full projection partial
(all 3 head-pairs summed on-chip); the host adds the 2 group partials + bias.

vs v1:
- exp() emits fp8e4 (e4m3) probabilities directly, with a global logit shift
  C (softmax-invariant) keeping exp in e4m3's dynamic range.
- PV runs as fp8 DoubleRow matmuls (2 j-chunk k-tiles per instruction, 0.5
  cycles/row) with v = v_hi + v_lo error-feedback pair; a ones column in
  v_hi's 65th row gives Z for free. PE time for PV halves vs the fp32r M=65
  scheme.
- The projection contracts all 3 pairs on-chip (bf16) into one bf16 output;
  the host sums 2 partials instead of 6.
- exp is the span-setting engine (~205us ACT); the schedule keeps the ACT
  stream gapless: minimal lead-in (coarse single-DMA weight/x loads, x issued
  from the otherwise-idle ACT queue), PV interleaved with one jj lag so the
  S->exp chain never queues behind PV, and proj lagged one block.
"""

import os
import sys
from contextlib import ExitStack

if "/opt/trn_rl_repo" not in sys.path:
    sys.path.insert(0, "/opt/trn_rl_repo")

import numpy as np
import ml_dtypes

import concourse.bass as bass
import concourse.mybir as mybir
import concourse.tile as tile
from concourse import bass_utils

F32 = mybir.dt.float32
BF16 = mybir.dt.bfloat16
F8 = mybir.dt.float8e4
U8 = mybir.dt.uint8
DRMODE = mybir.MatmulPerfMode.DoubleRow

B, N, C = 4, 2048, 768
NH, D = 12, 64
SCALE = D ** -0.5
HPC = NH // 2          # heads per core
F = HPC * D            # 384 per-core features per projection
P = 128
CO = C // P            # 6 contraction chunks
FO = F // P            # 3 head pairs
NO = N // P            # 16 token chunks
NJJ = NO // 2          # 8 j-chunk pairs (DoubleRow k-tiles)
NCORES = 8
VSTRIDE = 80           # v8hi per-head stride (must be %16==0 for DR lhsT)

C_SHIFT = float(os.environ.get("KERNEL_C_SHIFT", "4.0"))
A_SCH = 8.0 / np.log(2.0)
B_SCH = 55.5
# j-chunk indices (0..15) whose exp runs on the DVE via corrected Schraudolph.
# Default empty: the bit-trick exp fails the 2e-2 gate even at 25% mixing.
_dve_js = os.environ.get("KERNEL_DVE_JS", "")
DVE_JS = set(int(x) for x in _dve_js.split(",") if x != "")

MM_DT = mybir.dt.float32r


def _d(ap):
    return ap.bitcast(MM_DT)


def _r(ap):
    """Cast a producer OUT AP feeding an fp32r matmul (rounding chain)."""
    return ap.bitcast(MM_DT)


def _split_multiwaits(nc):
    """Walrus accepts at most ONE sync-wait per instruction: split extras
    into single-wait NOPs queued just before (FIFO-equivalent)."""
    ctr = 0
    for f in nc.m.functions:
        for blk in f.blocks:
            insts = blk.instructions
            out = []
            changed = False
            for ins in insts:
                si = ins.sync_info
                if si is not None and len(si.on_wait) > 1:
                    changed = True
                    waits = list(si.on_wait)
                    for ww in waits[:-1]:
                        nop = mybir.InstNoOp(name=f"zzsplitw_{ctr}", ins=[], outs=[])
                        ctr += 1
                        nop.engine = ins.engine
                        nop.sync_info = mybir.SyncInfo(on_wait=[ww], on_update=[])
                        out.append(nop)
                    ins.sync_info = mybir.SyncInfo(
                        on_wait=[waits[-1]], on_update=list(si.on_update)
                    )
                out.append(ins)
            if changed:
                blk.instructions = out
    return nc


def _emit(nc, tc, ctx):
    # x n4-major: [n4][128, CO, 512] bf16 so one cheap DMA per 512-token slice
    xTn = nc.dram_tensor("xTn", [4, P, CO, 512], BF16, kind="ExternalInput").ap()
    # five contiguous weight sections [C, w]; loaded with ONE rearranged DMA
    # each into [128, CO, w] (HWDGE descgen is a serial device: fewer DMAs)
    SECS = ((0, P), (F, F + P), (2 * F, 3 * F), (P, F), (F + P, 2 * F))
    wq_secs = {
        lo: nc.dram_tensor(f"wq{lo}", [C, hi - lo], BF16, kind="ExternalInput").ap()
        for lo, hi in SECS
    }
    wprojT = nc.dram_tensor("wprojT", [FO, P, C], BF16, kind="ExternalInput").ap()
    out3 = nc.dram_tensor("out3", [N, C], BF16, kind="ExternalOutput").ap()

    persist = ctx.enter_context(tc.tile_pool(name="persist", bufs=1))

    ones_pe = persist.tile([P, 64], F32, tag="ones_pe")
    warm_in = persist.tile([64, 512], F32, tag="warm_in")
    nc.gpsimd.memset(warm_in, 0.0)
    nc.vector.memset(ones_pe, 1.0)

    # q/k [feature, token] fp32, per (pair, 512-token chunk)
    q_sb = [[persist.tile([P, 512], F32, tag=f"q{fo}_{n4}", name=f"q{fo}_{n4}")
             for n4 in range(4)] for fo in range(FO)]
    k_sb = [[persist.tile([P, 512], F32, tag=f"k{fo}_{n4}", name=f"k{fo}_{n4}")
             for n4 in range(4)] for fo in range(FO)]
    # v fp8 hi/lo per jj: [j-in-chunk, kt, head-strided columns]
    v8hi = [persist.tile([P, 2, HPC * VSTRIDE], F8, tag=f"v8hi{jj}", name=f"v8hi{jj}")
            for jj in range(NJJ)]
    v8lo = [persist.tile([P, 2, HPC * D], F8, tag=f"v8lo{jj}", name=f"v8lo{jj}")
            for jj in range(NJJ)]
    # attention output (normalized) per pair, [feature, token] bf16
    ot_sb = [persist.tile([P, N], BF16, tag=f"ot{pr}", name=f"ot{pr}")
             for pr in range(FO)]
    wp_sb = persist.tile([P, FO, C], BF16, tag="wp")
    # pair-2 odd-head rows of wp copied to partitions 0:64 (tail split-K proj
    # reads the normalized odd half from nt, which lives at partitions 0:64)
    wp2dn = persist.tile([64, C], BF16, tag="wp2dn")

    # ones row at partition 64 for the tail's PE-broadcast of 1/Z (memset
    # first: the PE warm-up dummies depend on it)
    ones_pe0 = None
    # ones columns of v8hi (Z rows)
    for jj in range(NJJ):
        nc.gpsimd.memset(
            v8hi[jj].rearrange("p t (h s) -> p t h s", s=VSTRIDE)[:, :, :, D : D + 1],
            1.0,
        )
    # bias AP for exp's global logit shift (softmax-invariant)
    bias_c = persist.tile([P, 1], F32, tag="biasc")
    nc.vector.memset(bias_c, -C_SHIFT)
    # dummy exp: pull the ACT table load into the DMA lead-in window
    expwarm = persist.tile([P, 4], F32, tag="expwarm")
    nc.vector.memset(expwarm, 0.0)
    nc.scalar.activation(
        out=expwarm, in_=expwarm, func=mybir.ActivationFunctionType.Exp, scale=1.0,
        bias=bias_c,
    )

    with (
        tc.tile_pool(name="wqp", bufs=1) as wqp,
        tc.tile_pool(name="xs", bufs=4) as xs_pool,
        tc.tile_pool(name="p8p", bufs=12) as p8_pool,
        tc.tile_pool(name="rp", bufs=2) as r_pool,
        tc.tile_pool(name="outp", bufs=2) as outp,
        tc.tile_pool(name="rd", bufs=3, space="DRAM") as rd_pool,
        tc.tile_pool(name="ps_st", bufs=2, space="PSUM") as ps_st,
        tc.tile_pool(name="ps_po", bufs=2, space="PSUM") as ps_po,
        tc.tile_pool(name="ps_mm", bufs=2, space="PSUM") as ps_mm,
    ):
        wq_tiles = {}

        def load_wq(lo, hi):
            w = hi - lo
            t = wqp.tile([P, CO, w], BF16, tag=f"wq_{lo}", name=f"wq_{lo}")
            wq_tiles[lo] = t
            nc.sync.dma_start(
                out=t,
                in_=wq_secs[lo].rearrange("(co p) w -> p co w", p=P),
            )

        def wq_slice(foff, co, width=P):
            for lo, hi in SECS:
                if lo <= foff and foff + width <= hi:
                    return wq_tiles[lo][:, co, foff - lo : foff - lo + width]
            raise KeyError(foff)

        # ---- QKV -------------------------------------------------------
        def emit_qk_group(fo, n4, which, xt4):
            """One q or k psum group [128, 512] -> q_sb/k_sb fp32 (bf16 mm)."""
            dst = (q_sb if which == "q" else k_sb)[fo][n4]
            foff = fo * P if which == "q" else F + fo * P
            pq = ps_mm.tile([P, 512], F32, tag="mm", name=f"p{which}{fo}_{n4}")
            for co in range(CO):
                nc.tensor.matmul(
                    pq, wq_slice(foff, co), xt4[:, co, :],
                    start=(co == 0), stop=(co == CO - 1),
                )
            nc.vector.tensor_copy(out=_r(dst), in_=pq)

        def emit_x_dma(n4):
            xt4 = xs_pool.tile([P, CO, 512], BF16, tag="xt4", name=f"xt4_{n4}")
            nc.sync.dma_start(out=xt4, in_=xTn[n4, :, :, :])
            return xt4

        def emit_v_chunk(no, xtv):
            """v for one 128-token chunk -> v8hi/v8lo fp8 (+ones col in hi)."""
            pv = ps_mm.tile([P, F], F32, tag="mm", name=f"pv_{no}")
            for co in range(CO):
                nc.tensor.matmul(
                    pv,
                    xtv[:, co, (no % 4) * P : (no % 4 + 1) * P],
                    wq_slice(2 * F, co, F),
                    start=(co == 0), stop=(co == CO - 1),
                )
            jj, half = divmod(no, 2)
            hi = v8hi[jj].rearrange("p t (h s) -> p t h s", s=VSTRIDE)
            nc.vector.tensor_copy(
                out=hi[:, half, :, 0:D],
                in_=pv.rearrange("p (h d) -> p h d", h=HPC),
            )
            lo = v8lo[jj].rearrange("p t (h d) -> p t h d", d=D)
            nc.vector.tensor_sub(
                out=lo[:, half, :, :],
                in0=pv.rearrange("p (h d) -> p h d", h=HPC),
                in1=hi[:, half, :, 0:D],
            )

        # ---- attention -------------------------------------------------
        def emit_pv(pr, jj, p8, po_a, po_b):
            hA, hB = 2 * pr, 2 * pr + 1
            vh = v8hi[jj]
            vl = v8lo[jj]
            for h, po, icol in ((hA, po_a, 0), (hB, po_b, 512)):
                nc.tensor.matmul(
                    po,
                    vh[:, :, h * VSTRIDE : h * VSTRIDE + 65],
                    p8[:, :, icol : icol + 512],
                    start=(jj == 0), stop=False,
                    perf_mode=DRMODE,
                    skip_group_check=True,
                )
                nc.tensor.matmul(
                    po[0:64, :],
                    vl[:, :, h * D : h * D + 64],
                    p8[:, :, icol : icol + 512],
                    start=False, stop=(jj == NJJ - 1),
                    perf_mode=DRMODE,
                    skip_group_check=True,
                )

        def make_norm(pr, i4, po_a, po_b, pe_bcast=False):
            """Closure emitting the normalize chain for block (pr, i4).

            pe_bcast: broadcast 1/Z across partitions with a K=1 ones-matmul
            into a free PSUM slot instead of the DRAM bounce (tail only --
            saves the DMA round trip and keeps the PE warm for proj).
            """
            i0 = i4 * 512

            def norm_pe():
                rv = r_pool.tile([65, 1024], F32, tag="rv", name=f"rv_{pr}_{i4}")
                with nc.allow_low_precision(reason="1/Z feeds fp32r bcast matmul"):
                    nc.vector.reciprocal(out=_r(rv[64:65, 0:512]), in_=po_a[64:65, :])
                    nc.vector.reciprocal(out=_r(rv[64:65, 512:1024]), in_=po_b[64:65, :])
                rb_ps = ps_st.tile([64, 1024], F32, tag="st", name=f"rbps_{pr}_{i4}")
                for h in range(2):
                    nc.tensor.matmul(
                        rb_ps[:, h * 512 : h * 512 + 512],
                        _d(ones_pe[64:65, :]),
                        _d(rv[64:65, h * 512 : h * 512 + 512]),
                        start=True, stop=True,
                        tile_position=(64, 0),
                    )
                rb_sb = r_pool.tile([64, 2, 512], F32, tag="rb", name=f"rbs_{pr}_{i4}")
                nc.vector.tensor_copy(out=rb_sb, in_=rb_ps.rearrange("p (t n) -> p t n", t=2))
                nc.vector.tensor_mul(
                    out=ot_sb[pr][0:64, i0 : i0 + 512], in0=po_a[0:64, :],
                    in1=rb_sb[:, 0, :],
                )
                nt = r_pool.tile([64, 512], BF16, tag="nt", name=f"nt_{pr}_{i4}")
                nc.vector.tensor_mul(out=nt, in0=po_b[0:64, :], in1=rb_sb[:, 1, :])
                nc.sync.dma_start(out=ot_sb[pr][64:128, i0 : i0 + 512], in_=nt)
                tail_nt["nt"] = nt

            def norm():
                if pe_bcast:
                    return norm_pe()
                rv = r_pool.tile([65, 1024], F32, tag="rv", name=f"rv_{pr}_{i4}")
                nc.vector.reciprocal(out=rv[64:65, 0:512], in_=po_a[64:65, :])
                nc.vector.reciprocal(out=rv[64:65, 512:1024], in_=po_b[64:65, :])
                rdram = rd_pool.tile([1, 1024], F32, tag="rd", name=f"rd_{pr}_{i4}")
                nc.sync.dma_start(out=rdram, in_=rv[64:65, :])
                rb = r_pool.tile([64, 2, 512], F32, tag="rb", name=f"rb_{pr}_{i4}")
                nc.sync.dma_start(out=rb[:, 0, :], in_=rdram[0:1, 0:512].to_broadcast([64, 512]))
                nc.sync.dma_start(out=rb[:, 1, :], in_=rdram[0:1, 512:1024].to_broadcast([64, 512]))
                # multiply straight out of PSUM (DVE reads psum; po slot is
                # not needed again until ~a block later)
                nc.vector.tensor_mul(
                    out=ot_sb[pr][0:64, i0 : i0 + 512], in0=po_a[0:64, :], in1=rb[:, 0, :],
                )
                nt = r_pool.tile([64, 512], BF16, tag="nt", name=f"nt_{pr}_{i4}")
                nc.vector.tensor_mul(out=nt, in0=po_b[0:64, :], in1=rb[:, 1, :])
                nc.sync.dma_start(out=ot_sb[pr][64:128, i0 : i0 + 512], in_=nt)

            return norm

        def emit_attention(pr, interleave=None, cascade=False, carry_in=None):
            """Attention for pair pr over 4 i-blocks of 512.

            cascade=False: PV(jj) inline with a one-jj lag (j = 2jj+3), last
            PV + normalize at block end (pairs 1, 2 -- PE fits in ACT slack).
            cascade=True: block k's 16 PV instructions + its normalize run
            spread inside block k+1's j-loop (pair 0 -- makes room for the v
            projection inside blocks 0-1 without stalling the exp stream).
            The last block's closures are returned as carry for the next
            pair's first block.
            """
            pending = list(carry_in or [])

            def pop_pending():
                if pending:
                    pending.pop(0)()

            for i4 in range(4):
                po_a = ps_po.tile([65, 512], F32, tag="po", name=f"poA_{pr}_{i4}")
                po_b = ps_po.tile([65, 512], F32, tag="po", name=f"poB_{pr}_{i4}")
                p8s = []
                for j in range(NO):
                    kt = k_sb[pr][j // 4]
                    jo = (j % 4) * P
                    qt = q_sb[pr][i4]
                    stm = ps_st.tile([P, 1024], F32, tag="st", name=f"st_{pr}_{i4}_{j}")
                    nc.tensor.matmul(
                        stm[:, 0:512], _d(kt[0:64, jo : jo + P]), _d(qt[0:64, :]),
                        start=True, stop=True,
                    )
                    nc.tensor.matmul(
                        stm[:, 512:1024], _d(kt[64:128, jo : jo + P]), _d(qt[64:128, :]),
                        start=True, stop=True,
                    )
                    if j % 2 == 0:
                        p8 = p8_pool.tile([P, 2, 1024], F8, tag="p8",
                                          name=f"p8_{pr}_{i4}_{j // 2}")
                        p8s.append(p8)
                    p8 = p8s[j // 2]
                    if j in DVE_JS:
                        nc.vector.tensor_scalar(
                            out=p8[:, j % 2, :].bitcast(U8),
                            in0=stm,
                            scalar1=SCALE * A_SCH,
                            scalar2=B_SCH - C_SHIFT * A_SCH,
                            op0=mybir.AluOpType.mult,
                            op1=mybir.AluOpType.add,
                        )
                    else:
                        nc.scalar.activation(
                            out=p8[:, j % 2, :],
                            in_=stm,
                            func=mybir.ActivationFunctionType.Exp,
                            scale=SCALE,
                            bias=bias_c,
                        )
                    if interleave is not None:
                        interleave(i4, j)
                    if cascade:
                        # one pending item per odd j: PV(jj) lands at j=2jj+1,
                        # safely after v(2jj+1)'s write at even j
                        if j % 2 == 1:
                            pop_pending()
                    elif j >= 7 and j % 2 == 1:
                        emit_pv(pr, (j - 7) // 2, p8s[(j - 7) // 2], po_a, po_b)
                if cascade:
                    while pending:
                        pop_pending()
                    for jj in range(NJJ):
                        _jj, _p8 = jj, p8s[jj]
                        pending.append(
                            lambda _jj=_jj, _p8=_p8, _pa=po_a, _pb=po_b: emit_pv(
                                pr, _jj, _p8, _pa, _pb
                            )
                        )
                    pending.append(make_norm(pr, i4, po_a, po_b))
                else:
                    emit_pv(pr, NJJ - 3, p8s[NJJ - 3], po_a, po_b)
                    emit_pv(pr, NJJ - 2, p8s[NJJ - 2], po_a, po_b)
                    emit_pv(pr, NJJ - 1, p8s[NJJ - 1], po_a, po_b)
                    make_norm(pr, i4, po_a, po_b,
                              pe_bcast=(pr == 2 and i4 == 3))()
            return pending

        # ---- projection (all 3 pairs on-chip, bf16) --------------------
        proj_state = {"o_sb": None}
        tail_nt = {}

        def emit_proj(no_range, evac_act=False):
            for no in no_range:
                if no % 2 == 0:
                    proj_state["o_sb"] = outp.tile(
                        [P, 2, C], BF16, tag="o", name=f"o_{no}"
                    )
                o_sb = proj_state["o_sb"]
                for half in range(2):
                    pp = ps_mm.tile([P, 384], F32, tag="mm", name=f"pp_{no}_{half}")
                    for fo in range(FO):
                        nc.tensor.matmul(
                            pp,
                            ot_sb[fo][:, no * P : (no + 1) * P],
                            wp_sb[:, fo, half * 384 : half * 384 + 384],
                            start=(fo == 0), stop=(fo == FO - 1),
                        )
                    dst = o_sb[:, no % 2, half * 384 : half * 384 + 384]
                    if evac_act and half == 0:
                        nc.scalar.copy(out=dst, in_=pp)
                    else:
                        nc.vector.tensor_copy(out=dst, in_=pp)
                if no % 2 == 1:
                    nc.sync.dma_start(
                        out=out3[(no - 1) * P : (no + 1) * P, :].rearrange(
                            "(t p) c -> p t c", t=2
                        ),
                        in_=o_sb,
                    )

        # ---- schedule --------------------------------------------------
        # lead-in (bf16 DMAs pipeline with the k-group chain): wqk, x0, wq0,
        # x1, wv, x2, x3 -- k(0,n4) lands just before the exp stream needs it
        xt4_0 = xs_pool.tile([P, CO, 512], BF16, tag="xt4", name="xt4_0")
        nc.sync.dma_start(out=xt4_0[:, 0, :], in_=xTn[0, :, 0, :])
        nc.sync.dma_start(out=xt4_0[:, 1, :], in_=xTn[0, :, 1, :])
        load_wq(F, F + P)                               # k0 weights
        nc.sync.dma_start(out=xt4_0[:, 2, :], in_=xTn[0, :, 2, :])
        load_wq(0, P)                                   # q0 weights
        for co in range(3, CO):
            nc.sync.dma_start(out=xt4_0[:, co, :], in_=xTn[0, :, co, :])
        xt4s0 = [xt4_0]
        # warm the PE clock while the lead-in DMAs stream (HAM ramp)
        warm_sc = ps_st.tile([64, 1024], F32, tag="st", name="warm_sc")
        for _ in range(10):
            nc.tensor.matmul(
                warm_sc[:, 0:128],
                _d(warm_in[:, 0:64]),
                _d(warm_in[:, 0:128]),
                start=True, stop=True,
            )
        xt4s0.append(emit_x_dma(1))
        load_wq(2 * F, 3 * F)                           # v weights
        xt4s0.append(emit_x_dma(2))
        xt4s0.append(emit_x_dma(3))
        # fused k(0,0)+q(0,0): co-matmuls pipelined against x0 chunk arrivals
        pk0 = ps_mm.tile([P, 512], F32, tag="mm", name="pk00")
        pq0 = ps_mm.tile([P, 512], F32, tag="mm", name="pq00")
        for co in range(CO):
            nc.tensor.matmul(
                pk0, wq_slice(F, co), xt4_0[:, co, :],
                start=(co == 0), stop=(co == CO - 1),
            )
            nc.tensor.matmul(
                pq0, wq_slice(0, co), xt4_0[:, co, :],
                start=(co == 0), stop=(co == CO - 1),
            )
        nc.vector.tensor_copy(out=_r(k_sb[0][0]), in_=pk0)
        nc.vector.tensor_copy(out=_r(q_sb[0][0]), in_=pq0)
        nc.sync.dma_start(
            out=wp_sb, in_=wprojT.rearrange("fo p c -> p fo c"),
        )
        nc.sync.dma_start(out=wp2dn, in_=wprojT[2, 64:128, :])
        load_wq(P, F)                                   # q1/q2
        load_wq(F + P, 2 * F)                           # k1/k2

        # pair 0 (cascade PV): v chunks spread over blocks 0-1, q(0,i4+1)
        # at j==14, pair-1 qkv groups in blocks 2-3
        x_cache = {}
        qk1_slots = [(2, 2), (2, 6), (2, 10), (3, 2), (3, 4), (3, 8), (3, 10), (3, 12)]
        qk2_slots = [(0, 4), (0, 10), (1, 4), (1, 10), (2, 4), (2, 10), (3, 4), (3, 10)]
        qkn4 = [0, 1, 2, 3, 0, 1, 2, 3]

        def emit_qk_pair_group(pr, idx):
            n4 = qkn4[idx]
            if (pr, n4) not in x_cache:
                if pr == 2:
                    x_cache[(pr, n4)] = x_cache[(1, n4)]
                else:
                    x_cache[(pr, n4)] = emit_x_dma(n4)
            xt4 = x_cache[(pr, n4)]
            emit_qk_group(pr, n4, "q" if idx < 4 else "k", xt4)

        def inter0(i4, j):
            # k(0,1..3) pipelined against the x1..x3 DMA arrivals
            if i4 == 0 and j in (1, 5, 9):
                emit_qk_group(0, 1 + (j - 1) // 4, "k", xt4s0[1 + (j - 1) // 4])
            # v(no): 10 chunks late in block 0 (after wv lands), 6 in block 1
            if i4 == 0 and 6 <= j:
                emit_v_chunk(j - 6, xt4s0[(j - 6) // 4])
            elif i4 == 1 and j % 2 == 0 and 2 <= j <= 12:
                emit_v_chunk(10 + (j - 2) // 2, xt4s0[(10 + (j - 2) // 2) // 4])
            if j == 14 and i4 < 3:
                emit_qk_group(0, i4 + 1, "q", xt4s0[i4 + 1])
            if (i4, j) in qk1_slots:
                emit_qk_pair_group(1, qk1_slots.index((i4, j)))

        carry = emit_attention(0, interleave=inter0, cascade=True)

        def inter1(i4, j):
            if (i4, j) in qk2_slots:
                emit_qk_pair_group(2, qk2_slots.index((i4, j)))
            # drain pair-0 block-3's carried PVs + norm in block 0
            if i4 == 0 and j % 2 == 1 and carry:
                carry.pop(0)()
                if j == 13:
                    while carry:
                        carry.pop(0)()

        emit_attention(1, interleave=inter1)

        # attention 2: proj for block i4-1's chunks lands inside block i4
        def inter2(i4, j):
            if i4 >= 1 and j in (5, 8, 11, 14):
                no = 4 * (i4 - 1) + (5, 8, 11, 14).index(j)
                emit_proj([no])

        emit_attention(2, interleave=inter2)
        for _ in range(10):
            nc.tensor.matmul(
                warm_sc[:, 0:64], _d(ones_pe[0:64, :]), _d(ones_pe[0:64, :]),
                start=True, stop=True,
            )
        # tail proj: split pair-2's contraction so the odd-head half reads nt
        # (SBUF partitions 0:64) instead of waiting for the ot DMA-up
        nt3 = tail_nt["nt"]
        for no in range(12, 16):
            if no % 2 == 0:
                proj_state["o_sb"] = outp.tile([P, 2, C], BF16, tag="o", name=f"o_{no}")
            o_sb = proj_state["o_sb"]
            for half in range(2):
                pp = ps_mm.tile([P, 384], F32, tag="mm", name=f"pp_{no}_{half}")
                hc = slice(half * 384, half * 384 + 384)
                for fo in range(2):
                    nc.tensor.matmul(
                        pp, ot_sb[fo][:, no * P : (no + 1) * P], wp_sb[:, fo, hc],
                        start=(fo == 0), stop=False, skip_group_check=True,
                    )
                nc.tensor.matmul(
                    pp, ot_sb[2][0:64, no * P : (no + 1) * P], wp_sb[0:64, 2, hc],
                    start=False, stop=False, skip_group_check=True,
                )
                nc.tensor.matmul(
                    pp, nt3[:, (no % 4) * P : (no % 4 + 1) * P], wp2dn[:, hc],
                    start=False, stop=True, skip_group_check=True,
                )
                dst = o_sb[:, no % 2, hc]
                if half == 0:
                    nc.scalar.copy(out=dst, in_=pp)
                else:
                    nc.vector.tensor_copy(out=dst, in_=pp)
            if no >= 14:
                nc.sync.dma_start(
                    out=out3[no * P : (no + 1) * P, :],
                    in_=proj_state["o_sb"][:, no % 2, :],
                )
            elif no % 2 == 1:
                nc.sync.dma_start(
                    out=out3[(no - 1) * P : (no + 1) * P, :].rearrange(
                        "(t p) c -> p t c", t=2
                    ),
                    in_=proj_state["o_sb"],
                )


_NC_CACHE = {}


def build_bass():
    key = (C_SHIFT, tuple(sorted(DVE_JS)))
    if key in _NC_CACHE:
        return _NC_CACHE[key]
    nc = bass.Bass("TRN2")
    with tile.TileContext(nc) as tc:
        with ExitStack() as ctx:
            _emit(nc, tc, ctx)
    _split_multiwaits(nc)
    _NC_CACHE[key] = nc
    return nc


def make_in_maps(x, w_qkv, w_proj):
    x = np.asarray(x, dtype=np.float32)
    w_qkv = np.asarray(w_qkv, dtype=np.float32)
    w_proj = np.asarray(w_proj, dtype=np.float32)
    wq, wk, wv = w_qkv[0:C], w_qkv[C : 2 * C], w_qkv[2 * C : 3 * C]
    in_maps = []
    for c in range(NCORES):
        b, g = divmod(c, 2)
        sl = slice(g * F, (g + 1) * F)
        wslice = np.concatenate([wq[sl], wk[sl], wv[sl]], axis=0)  # [1152, 768]
        wT = np.ascontiguousarray(wslice.T)  # [768, 1152]
        xT = x[b].T  # [768, 2048]
        # [n4][128, co, 512] bf16
        xTn = np.ascontiguousarray(
            xT.reshape(CO, P, 4, 512).transpose(2, 1, 0, 3).astype(ml_dtypes.bfloat16)
        )
        wpT = np.ascontiguousarray(w_proj[:, sl].T)  # [384, 768]
        m = {
            "xTn": xTn,
            "wprojT": np.ascontiguousarray(
                wpT.reshape(FO, P, C).astype(ml_dtypes.bfloat16)
            ),
        }
        for lo, hi in ((0, 128), (384, 512), (768, 1152), (128, 384), (512, 768)):
            m[f"wq{lo}"] = np.ascontiguousarray(wT[:, lo:hi].astype(ml_dtypes.bfloat16))
        in_maps.append(m)
    return in_maps


def gather_output(parts, b_proj):
    outv = np.empty((B, N, C), np.float32)
    for b in range(B):
        outv[b] = parts[2 * b].astype(np.float32) + parts[2 * b + 1].astype(np.float32)
    outv += np.asarray(b_proj, dtype=np.float32)[None, None, :]
    return outv


def kernel(x, w_qkv, w_proj, b_proj, _run_kwargs=None):
    nc = build_bass()
    in_maps = make_in_maps(x, w_qkv, w_proj)
    res = bass_utils.run_bass_kernel_spmd(
        nc, in_maps, core_ids=list(range(NCORES)), **(_run_kwargs or {})
    )
    parts = [r["out3"] for r in res.results]
    outv = gather_output(parts, b_proj)
    if _run_kwargs is not None:
        kernel.last_results = res
    return outv


# revision 6
# speedup vs baseline: 1.0046x; 1.0031x over previous
"""Trainium2 Bass kernel v2 for nn_Attention (B=4, N=2048, C=768, H=12).

Sharding: 8 cores = 4 batches x 2 head-groups (6 heads each). Each core
computes qkv for its 6 heads, attention, and the full projection partial
(all 3 head-pairs summed on-chip); the host adds the 2 group partials + bias.

vs v1:
- exp() emits fp8e4 (e4m3) probabilities directly, with a global logit shift
  C (softmax-invariant) keeping exp in e4m3's dynamic range.
- PV runs as fp8 DoubleRow matmuls (2 j-chunk k-tiles per instruction, 0.5
  cycles/row) with v = v_hi + v_lo error-feedback pair; a ones column in
  v_hi's 65th row gives Z for free. PE time for PV halves vs the fp32r M=65
  scheme.
- The projection contracts all 3 pairs on-chip (bf16) into one bf16 output;
  the host sums 2 partials instead of 6.
- exp is the span-setting engine (~205us ACT); the schedule keeps the ACT
  stream gapless: minimal lead-in (coarse single-DMA weight/x loads, x issued
  from the otherwise-idle ACT queue), PV interleaved with one jj lag so the
  S->exp chain never queues behind PV, and proj lagged one block.
"""

import os
import sys
from contextlib import ExitStack

if "/opt/trn_rl_repo" not in sys.path:
    sys.path.insert(0, "/opt/trn_rl_repo")

import numpy as np
import ml_dtypes

import concourse.bass as bass
import concourse.mybir as mybir
import concourse.tile as tile
from concourse import bass_utils

F32 = mybir.dt.float32
BF16 = mybir.dt.bfloat16
F8 = mybir.dt.float8e4
U8 = mybir.dt.uint8
DRMODE = mybir.MatmulPerfMode.DoubleRow

B, N, C = 4, 2048, 768
NH, D = 12, 64
SCALE = D ** -0.5
HPC = NH // 2          # heads per core
F = HPC * D            # 384 per-core features per projection
P = 128
CO = C // P            # 6 contraction chunks
FO = F // P            # 3 head pairs
NO = N // P            # 16 token chunks
NJJ = NO // 2          # 8 j-chunk pairs (DoubleRow k-tiles)
NCORES = 8
VSTRIDE = 80           # v8hi per-head stride (must be %16==0 for DR lhsT)

C_SHIFT = float(os.environ.get("KERNEL_C_SHIFT", "4.0"))
A_SCH = 8.0 / np.log(2.0)
B_SCH = 55.5
# j-chunk indices (0..15) whose exp runs on the DVE via corrected Schraudolph.
# Default empty: the bit-trick exp fails the 2e-2 gate even at 25% mixing.
_dve_js = os.environ.get("KERNEL_DVE_JS", "")
DVE_JS = set(int(x) for x in _dve_js.split(",") if x != "")

MM_DT = mybir.dt.float32r


def _d(ap):
    return ap.bitcast(MM_DT)


def _r(ap):
    """Cast a producer OUT AP feeding an fp32r matmul (rounding chain)."""
    return ap.bitcast(MM_DT)


def _split_multiwaits(nc):
    """Walrus accepts at most ONE sync-wait per instruction: split extras
    into single-wait NOPs queued just before (FIFO-equivalent)."""
    ctr = 0
    for f in nc.m.functions:
        for blk in f.blocks:
            insts = blk.instructions
            out = []
            changed = False
            for ins in insts:
                si = ins.sync_info
                if si is not None and len(si.on_wait) > 1:
                    changed = True
                    waits = list(si.on_wait)
                    for ww in waits[:-1]:
                        nop = mybir.InstNoOp(name=f"zzsplitw_{ctr}", ins=[], outs=[])
                        ctr += 1
                        nop.engine = ins.engine
                        nop.sync_info = mybir.SyncInfo(on_wait=[ww], on_update=[])
                        out.append(nop)
                    ins.sync_info = mybir.SyncInfo(
                        on_wait=[waits[-1]], on_update=list(si.on_update)
                    )
                out.append(ins)
            if changed:
                blk.instructions = out
    return nc


def _emit(nc, tc, ctx):
    # x n4-major: [n4][128, CO, 512] bf16 so one cheap DMA per 512-token slice
    xTn = nc.dram_tensor("xTn", [4, P, CO, 512], BF16, kind="ExternalInput").ap()
    # five contiguous weight sections [C, w]; loaded with ONE rearranged DMA
    # each into [128, CO, w] (HWDGE descgen is a serial device: fewer DMAs)
    SECS = ((0, P), (F, F + P), (2 * F, 3 * F), (P, F), (F + P, 2 * F))
    wq_secs = {
        lo: nc.dram_tensor(f"wq{lo}", [C, hi - lo], BF16, kind="ExternalInput").ap()
        for lo, hi in SECS
    }
    wprojT = nc.dram_tensor("wprojT", [FO, P, C], BF16, kind="ExternalInput").ap()
    out3 = nc.dram_tensor("out3", [N, C], BF16, kind="ExternalOutput").ap()

    persist = ctx.enter_context(tc.tile_pool(name="persist", bufs=1))

    ones_pe = persist.tile([P, 64], F32, tag="ones_pe")
    warm_in = persist.tile([64, 512], F32, tag="warm_in")
    nc.gpsimd.memset(warm_in, 0.0)
    nc.vector.memset(ones_pe, 1.0)

    # q/k [feature, token] fp32, per (pair, 512-token chunk)
    q_sb = [[persist.tile([P, 512], F32, tag=f"q{fo}_{n4}", name=f"q{fo}_{n4}")
             for n4 in range(4)] for fo in range(FO)]
    k_sb = [[persist.tile([P, 512], F32, tag=f"k{fo}_{n4}", name=f"k{fo}_{n4}")
             for n4 in range(4)] for fo in range(FO)]
    # v fp8 hi/lo per jj: [j-in-chunk, kt, head-strided columns]
    v8hi = [persist.tile([P, 2, HPC * VSTRIDE], F8, tag=f"v8hi{jj}", name=f"v8hi{jj}")
            for jj in range(NJJ)]
    v8lo = [persist.tile([P, 2, HPC * D], F8, tag=f"v8lo{jj}", name=f"v8lo{jj}")
            for jj in range(NJJ)]
    # attention output (normalized) per pair, [feature, token] bf16
    ot_sb = [persist.tile([P, N], BF16, tag=f"ot{pr}", name=f"ot{pr}")
             for pr in range(FO)]
    wp_sb = persist.tile([P, FO, C], BF16, tag="wp")
    # pair-2 odd-head rows of wp copied to partitions 0:64 (tail split-K proj
    # reads the normalized odd half from nt, which lives at partitions 0:64)
    wp2dn = persist.tile([64, C], BF16, tag="wp2dn")

    # ones row at partition 64 for the tail's PE-broadcast of 1/Z (memset
    # first: the PE warm-up dummies depend on it)
    ones_pe0 = None
    # ones columns of v8hi (Z rows)
    for jj in range(NJJ):
        nc.gpsimd.memset(
            v8hi[jj].rearrange("p t (h s) -> p t h s", s=VSTRIDE)[:, :, :, D : D + 1],
            1.0,
        )
    # bias AP for exp's global logit shift (softmax-invariant)
    bias_c = persist.tile([P, 1], F32, tag="biasc")
    nc.vector.memset(bias_c, -C_SHIFT)
    # dummy exp: pull the ACT table load into the DMA lead-in window
    expwarm = persist.tile([P, 4], F32, tag="expwarm")
    nc.vector.memset(expwarm, 0.0)
    nc.scalar.activation(
        out=expwarm, in_=expwarm, func=mybir.ActivationFunctionType.Exp, scale=1.0,
        bias=bias_c,
    )

    with (
        tc.tile_pool(name="wqp", bufs=1) as wqp,
        tc.tile_pool(name="xs", bufs=4) as xs_pool,
        tc.tile_pool(name="p8p", bufs=12) as p8_pool,
        tc.tile_pool(name="rp", bufs=2) as r_pool,
        tc.tile_pool(name="outp", bufs=2) as outp,
        tc.tile_pool(name="rd", bufs=3, space="DRAM") as rd_pool,
        tc.tile_pool(name="ps_st", bufs=2, space="PSUM") as ps_st,
        tc.tile_pool(name="ps_po", bufs=2, space="PSUM") as ps_po,
        tc.tile_pool(name="ps_mm", bufs=2, space="PSUM") as ps_mm,
    ):
        wq_tiles = {}

        def load_wq(lo, hi):
            w = hi - lo
            t = wqp.tile([P, CO, w], BF16, tag=f"wq_{lo}", name=f"wq_{lo}")
            wq_tiles[lo] = t
            nc.sync.dma_start(
                out=t,
                in_=wq_secs[lo].rearrange("(co p) w -> p co w", p=P),
            )

        def wq_slice(foff, co, width=P):
            for lo, hi in SECS:
                if lo <= foff and foff + width <= hi:
                    return wq_tiles[lo][:, co, foff - lo : foff - lo + width]
            raise KeyError(foff)

        # ---- QKV -------------------------------------------------------
        def emit_qk_group(fo, n4, which, xt4):
            """One q or k psum group [128, 512] -> q_sb/k_sb fp32 (bf16 mm)."""
            dst = (q_sb if which == "q" else k_sb)[fo][n4]
            foff = fo * P if which == "q" else F + fo * P
            pq = ps_mm.tile([P, 512], F32, tag="mm", name=f"p{which}{fo}_{n4}")
            for co in range(CO):
                nc.tensor.matmul(
                    pq, wq_slice(foff, co), xt4[:, co, :],
                    start=(co == 0), stop=(co == CO - 1),
                )
            nc.vector.tensor_copy(out=_r(dst), in_=pq)

        def emit_x_dma(n4):
            xt4 = xs_pool.tile([P, CO, 512], BF16, tag="xt4", name=f"xt4_{n4}")
            nc.sync.dma_start(out=xt4, in_=xTn[n4, :, :, :])
            return xt4

        def emit_v_chunk(no, xtv):
            """v for one 128-token chunk -> v8hi/v8lo fp8 (+ones col in hi)."""
            pv = ps_mm.tile([P, F], F32, tag="mm", name=f"pv_{no}")
            for co in range(CO):
                nc.tensor.matmul(
                    pv,
                    xtv[:, co, (no % 4) * P : (no % 4 + 1) * P],
                    wq_slice(2 * F, co, F),
                    start=(co == 0), stop=(co == CO - 1),
                )
            jj, half = divmod(no, 2)
            hi = v8hi[jj].rearrange("p t (h s) -> p t h s", s=VSTRIDE)
            nc.vector.tensor_copy(
                out=hi[:, half, :, 0:D],
                in_=pv.rearrange("p (h d) -> p h d", h=HPC),
            )
            lo = v8lo[jj].rearrange("p t (h d) -> p t h d", d=D)
            nc.vector.tensor_sub(
                out=lo[:, half, :, :],
                in0=pv.rearrange("p (h d) -> p h d", h=HPC),
                in1=hi[:, half, :, 0:D],
            )

        # ---- attention -------------------------------------------------
        def emit_pv(pr, jj, p8, po_a, po_b):
            hA, hB = 2 * pr, 2 * pr + 1
            vh = v8hi[jj]
            vl = v8lo[jj]
            for h, po, icol in ((hA, po_a, 0), (hB, po_b, 512)):
                nc.tensor.matmul(
                    po,
                    vh[:, :, h * VSTRIDE : h * VSTRIDE + 65],
                    p8[:, :, icol : icol + 512],
                    start=(jj == 0), stop=False,
                    perf_mode=DRMODE,
                    skip_group_check=True,
                )
                nc.tensor.matmul(
                    po[0:64, :],
                    vl[:, :, h * D : h * D + 64],
                    p8[:, :, icol : icol + 512],
                    start=False, stop=(jj == NJJ - 1),
                    perf_mode=DRMODE,
                    skip_group_check=True,
                )

        def make_norm(pr, i4, po_a, po_b, pe_bcast=False):
            """Closure emitting the normalize chain for block (pr, i4).

            pe_bcast: broadcast 1/Z across partitions with a K=1 ones-matmul
            into a free PSUM slot instead of the DRAM bounce (tail only --
            saves the DMA round trip and keeps the PE warm for proj).
            """
            i0 = i4 * 512

            def norm_pe():
                rv = r_pool.tile([65, 1024], F32, tag="rv", name=f"rv_{pr}_{i4}")
                with nc.allow_low_precision(reason="1/Z feeds fp32r bcast matmul"):
                    nc.vector.reciprocal(out=_r(rv[64:65, 0:512]), in_=po_a[64:65, :])
                    nc.vector.reciprocal(out=_r(rv[64:65, 512:1024]), in_=po_b[64:65, :])
                rb_ps = ps_st.tile([64, 1024], F32, tag="st", name=f"rbps_{pr}_{i4}")
                for h in range(2):
                    nc.tensor.matmul(
                        rb_ps[:, h * 512 : h * 512 + 512],
                        _d(ones_pe[64:65, :]),
                        _d(rv[64:65, h * 512 : h * 512 + 512]),
                        start=True, stop=True,
                        tile_position=(64, 0),
                    )
                rb_sb = r_pool.tile([64, 2, 512], F32, tag="rb", name=f"rbs_{pr}_{i4}")
                nc.vector.tensor_copy(out=rb_sb, in_=rb_ps.rearrange("p (t n) -> p t n", t=2))
                nc.vector.tensor_mul(
                    out=ot_sb[pr][0:64, i0 : i0 + 512], in0=po_a[0:64, :],
                    in1=rb_sb[:, 0, :],
                )
                nt = r_pool.tile([64, 512], BF16, tag="nt", name=f"nt_{pr}_{i4}")
                nc.vector.tensor_mul(out=nt, in0=po_b[0:64, :], in1=rb_sb[:, 1, :])
                nc.sync.dma_start(out=ot_sb[pr][64:128, i0 : i0 + 512], in_=nt)
                tail_nt["nt"] = nt

            def norm():
                if pe_bcast:
                    return norm_pe()
                rv = r_pool.tile([65, 1024], F32, tag="rv", name=f"rv_{pr}_{i4}")
                nc.vector.reciprocal(out=rv[64:65, 0:512], in_=po_a[64:65, :])
                nc.vector.reciprocal(out=rv[64:65, 512:1024], in_=po_b[64:65, :])
                rdram = rd_pool.tile([1, 1024], F32, tag="rd", name=f"rd_{pr}_{i4}")
                nc.sync.dma_start(out=rdram, in_=rv[64:65, :])
                rb = r_pool.tile([64, 2, 512], F32, tag="rb", name=f"rb_{pr}_{i4}")
                nc.sync.dma_start(out=rb[:, 0, :], in_=rdram[0:1, 0:512].to_broadcast([64, 512]))
                nc.sync.dma_start(out=rb[:, 1, :], in_=rdram[0:1, 512:1024].to_broadcast([64, 512]))
                # multiply straight out of PSUM (DVE reads psum; po slot is
                # not needed again until ~a block later)
                nc.vector.tensor_mul(
                    out=ot_sb[pr][0:64, i0 : i0 + 512], in0=po_a[0:64, :], in1=rb[:, 0, :],
                )
                nt = r_pool.tile([64, 512], BF16, tag="nt", name=f"nt_{pr}_{i4}")
                nc.vector.tensor_mul(out=nt, in0=po_b[0:64, :], in1=rb[:, 1, :])
                nc.sync.dma_start(out=ot_sb[pr][64:128, i0 : i0 + 512], in_=nt)

            return norm

        def emit_attention(pr, interleave=None, cascade=False, carry_in=None):
            """Attention for pair pr over 4 i-blocks of 512.

            cascade=False: PV(jj) inline with a one-jj lag (j = 2jj+3), last
            PV + normalize at block end (pairs 1, 2 -- PE fits in ACT slack).
            cascade=True: block k's 16 PV instructions + its normalize run
            spread inside block k+1's j-loop (pair 0 -- makes room for the v
            projection inside blocks 0-1 without stalling the exp stream).
            The last block's closures are returned as carry for the next
            pair's first block.
            """
            pending = list(carry_in or [])

            def pop_pending():
                if pending:
                    pending.pop(0)()

            for i4 in range(4):
                po_a = ps_po.tile([65, 512], F32, tag="po", name=f"poA_{pr}_{i4}")
                po_b = ps_po.tile([65, 512], F32, tag="po", name=f"poB_{pr}_{i4}")
                p8s = []
                for j in range(NO):
                    kt = k_sb[pr][j // 4]
                    jo = (j % 4) * P
                    qt = q_sb[pr][i4]
                    stm = ps_st.tile([P, 1024], F32, tag="st", name=f"st_{pr}_{i4}_{j}")
                    nc.tensor.matmul(
                        stm[:, 0:512], _d(kt[0:64, jo : jo + P]), _d(qt[0:64, :]),
                        start=True, stop=True,
                    )
                    nc.tensor.matmul(
                        stm[:, 512:1024], _d(kt[64:128, jo : jo + P]), _d(qt[64:128, :]),
                        start=True, stop=True,
                    )
                    if j % 2 == 0:
                        p8 = p8_pool.tile([P, 2, 1024], F8, tag="p8",
                                          name=f"p8_{pr}_{i4}_{j // 2}")
                        p8s.append(p8)
                    p8 = p8s[j // 2]
                    if j in DVE_JS:
                        nc.vector.tensor_scalar(
                            out=p8[:, j % 2, :].bitcast(U8),
                            in0=stm,
                            scalar1=SCALE * A_SCH,
                            scalar2=B_SCH - C_SHIFT * A_SCH,
                            op0=mybir.AluOpType.mult,
                            op1=mybir.AluOpType.add,
                        )
                    else:
                        nc.scalar.activation(
                            out=p8[:, j % 2, :],
                            in_=stm,
                            func=mybir.ActivationFunctionType.Exp,
                            scale=SCALE,
                            bias=bias_c,
                        )
                    if interleave is not None:
                        interleave(i4, j)
                    if cascade:
                        # one pending item per odd j: PV(jj) lands at j=2jj+1,
                        # safely after v(2jj+1)'s write at even j
                        if j % 2 == 1:
                            pop_pending()
                    elif j >= 7 and j % 2 == 1:
                        emit_pv(pr, (j - 7) // 2, p8s[(j - 7) // 2], po_a, po_b)
                if cascade:
                    while pending:
                        pop_pending()
                    for jj in range(NJJ):
                        _jj, _p8 = jj, p8s[jj]
                        pending.append(
                            lambda _jj=_jj, _p8=_p8, _pa=po_a, _pb=po_b: emit_pv(
                                pr, _jj, _p8, _pa, _pb
                            )
                        )
                    pending.append(make_norm(pr, i4, po_a, po_b))
                else:
                    emit_pv(pr, NJJ - 3, p8s[NJJ - 3], po_a, po_b)
                    emit_pv(pr, NJJ - 2, p8s[NJJ - 2], po_a, po_b)
                    emit_pv(pr, NJJ - 1, p8s[NJJ - 1], po_a, po_b)
                    make_norm(pr, i4, po_a, po_b,
                              pe_bcast=(pr == 2 and i4 == 3))()
            return pending

        # ---- projection (all 3 pairs on-chip, bf16) --------------------
        proj_state = {"o_sb": None}
        tail_nt = {}

        def emit_proj(no_range, evac_act=False):
            for no in no_range:
                if no % 2 == 0:
                    proj_state["o_sb"] = outp.tile(
                        [P, 2, C], BF16, tag="o", name=f"o_{no}"
                    )
                o_sb = proj_state["o_sb"]
                for half in range(2):
                    pp = ps_mm.tile([P, 384], F32, tag="mm", name=f"pp_{no}_{half}")
                    for fo in range(FO):
                        nc.tensor.matmul(
                            pp,
                            ot_sb[fo][:, no * P : (no + 1) * P],
                            wp_sb[:, fo, half * 384 : half * 384 + 384],
                            start=(fo == 0), stop=(fo == FO - 1),
                        )
                    dst = o_sb[:, no % 2, half * 384 : half * 384 + 384]
                    if evac_act and half == 0:
                        nc.scalar.copy(out=dst, in_=pp)
                    else:
                        nc.vector.tensor_copy(out=dst, in_=pp)
                if no % 2 == 1:
                    nc.sync.dma_start(
                        out=out3[(no - 1) * P : (no + 1) * P, :].rearrange(
                            "(t p) c -> p t c", t=2
                        ),
                        in_=o_sb,
                    )

        # ---- schedule --------------------------------------------------
        # lead-in (bf16 DMAs pipeline with the k-group chain): wqk, x0, wq0,
        # x1, wv, x2, x3 -- k(0,n4) lands just before the exp stream needs it
        xt4_0 = xs_pool.tile([P, CO, 512], BF16, tag="xt4", name="xt4_0")
        nc.sync.dma_start(out=xt4_0[:, 0, :], in_=xTn[0, :, 0, :])
        nc.sync.dma_start(out=xt4_0[:, 1, :], in_=xTn[0, :, 1, :])
        load_wq(F, F + P)                               # k0 weights
        nc.sync.dma_start(out=xt4_0[:, 2, :], in_=xTn[0, :, 2, :])
        load_wq(0, P)                                   # q0 weights
        for co in range(3, CO):
            nc.sync.dma_start(out=xt4_0[:, co, :], in_=xTn[0, :, co, :])
        xt4s0 = [xt4_0]
        # warm the PE clock while the lead-in DMAs stream (HAM ramp)
        warm_sc = ps_st.tile([64, 1024], F32, tag="st", name="warm_sc")
        for _ in range(10):
            nc.tensor.matmul(
                warm_sc[:, 0:128],
                _d(warm_in[:, 0:64]),
                _d(warm_in[:, 0:128]),
                start=True, stop=True,
            )
        xt4s0.append(emit_x_dma(1))
        load_wq(2 * F, 3 * F)                           # v weights
        xt4s0.append(emit_x_dma(2))
        xt4s0.append(emit_x_dma(3))
        # fused k(0,0)+q(0,0): co-matmuls pipelined against x0 chunk arrivals
        pk0 = ps_mm.tile([P, 512], F32, tag="mm", name="pk00")
        pq0 = ps_mm.tile([P, 512], F32, tag="mm", name="pq00")
        for co in range(CO):
            nc.tensor.matmul(
                pk0, wq_slice(F, co), xt4_0[:, co, :],
                start=(co == 0), stop=(co == CO - 1),
            )
            nc.tensor.matmul(
                pq0, wq_slice(0, co), xt4_0[:, co, :],
                start=(co == 0), stop=(co == CO - 1),
            )
        nc.vector.tensor_copy(out=_r(k_sb[0][0]), in_=pk0)
        nc.vector.tensor_copy(out=_r(q_sb[0][0]), in_=pq0)
        nc.sync.dma_start(
            out=wp_sb, in_=wprojT.rearrange("fo p c -> p fo c"),
        )
        nc.sync.dma_start(out=wp2dn, in_=wprojT[2, 64:128, :])
        load_wq(P, F)                                   # q1/q2
        load_wq(F + P, 2 * F)                           # k1/k2

        # pair 0 (cascade PV): v chunks spread over blocks 0-1, q(0,i4+1)
        # at j==14, pair-1 qkv groups in blocks 2-3
        x_cache = {}
        qk1_slots = [(2, 2), (2, 6), (2, 10), (3, 2), (3, 4), (3, 8), (3, 10), (3, 12)]
        qk2_slots = [(0, 4), (0, 10), (1, 4), (1, 10), (2, 4), (2, 10), (3, 4), (3, 10)]
        qkn4 = [0, 1, 2, 3, 0, 1, 2, 3]

        def emit_qk_pair_group(pr, idx):
            n4 = qkn4[idx]
            if (pr, n4) not in x_cache:
                if pr == 2:
                    x_cache[(pr, n4)] = x_cache[(1, n4)]
                else:
                    x_cache[(pr, n4)] = emit_x_dma(n4)
            xt4 = x_cache[(pr, n4)]
            emit_qk_group(pr, n4, "q" if idx < 4 else "k", xt4)

        def inter0(i4, j):
            # k(0,1..3) pipelined against the x1..x3 DMA arrivals
            if i4 == 0 and j in (1, 5, 9):
                emit_qk_group(0, 1 + (j - 1) // 4, "k", xt4s0[1 + (j - 1) // 4])
            # v(no): 10 chunks late in block 0 (after wv lands), 6 in block 1
            if i4 == 0 and 6 <= j:
                emit_v_chunk(j - 6, xt4s0[(j - 6) // 4])
            elif i4 == 1 and j % 2 == 0 and 2 <= j <= 12:
                emit_v_chunk(10 + (j - 2) // 2, xt4s0[(10 + (j - 2) // 2) // 4])
            if i4 == 0 and j == 14:
                emit_qk_group(0, 1, "q", xt4s0[1])
            elif i4 in (1, 2) and j == 9:
                emit_qk_group(0, i4 + 1, "q", xt4s0[i4 + 1])
            if (i4, j) in qk1_slots:
                emit_qk_pair_group(1, qk1_slots.index((i4, j)))

        carry = emit_attention(0, interleave=inter0, cascade=True)

        def inter1(i4, j):
            if (i4, j) in qk2_slots:
                emit_qk_pair_group(2, qk2_slots.index((i4, j)))
            # drain pair-0 block-3's carried PVs + norm in block 0
            if i4 == 0 and j % 2 == 1 and carry:
                carry.pop(0)()
                if j == 13:
                    while carry:
                        carry.pop(0)()

        emit_attention(1, interleave=inter1)

        # attention 2: proj for block i4-1's chunks lands inside block i4
        def inter2(i4, j):
            if i4 >= 1 and j in (5, 8, 11, 14):
                no = 4 * (i4 - 1) + (5, 8, 11, 14).index(j)
                emit_proj([no])

        emit_attention(2, interleave=inter2)
        for _ in range(10):
            nc.tensor.matmul(
                warm_sc[:, 0:64], _d(ones_pe[0:64, :]), _d(ones_pe[0:64, :]),
                start=True, stop=True,
            )
        # tail proj: split pair-2's contraction so the odd-head half reads nt
        # (SBUF partitions 0:64) instead of waiting for the ot DMA-up
        nt3 = tail_nt["nt"]
        for no in range(12, 16):
            if no % 2 == 0:
                proj_state["o_sb"] = outp.tile([P, 2, C], BF16, tag="o", name=f"o_{no}")
            o_sb = proj_state["o_sb"]
            for half in range(2):
                pp = ps_mm.tile([P, 384], F32, tag="mm", name=f"pp_{no}_{half}")
                hc = slice(half * 384, half * 384 + 384)
                for fo in range(2):
                    nc.tensor.matmul(
                        pp, ot_sb[fo][:, no * P : (no + 1) * P], wp_sb[:, fo, hc],
                        start=(fo == 0), stop=False, skip_group_check=True,
                    )
                nc.tensor.matmul(
                    pp, ot_sb[2][0:64, no * P : (no + 1) * P], wp_sb[0:64, 2, hc],
                    start=False, stop=False, skip_group_check=True,
                )
                nc.tensor.matmul(
                    pp, nt3[:, (no % 4) * P : (no % 4 + 1) * P], wp2dn[:, hc],
                    start=False, stop=True, skip_group_check=True,
                )
                dst = o_sb[:, no % 2, hc]
                if half == 0:
                    nc.scalar.copy(out=dst, in_=pp)
                else:
                    nc.vector.tensor_copy(out=dst, in_=pp)
            if no >= 14:
                nc.sync.dma_start(
                    out=out3[no * P : (no + 1) * P, :],
                    in_=proj_state["o_sb"][:, no % 2, :],
                )
            elif no % 2 == 1:
                nc.sync.dma_start(
                    out=out3[(no - 1) * P : (no + 1) * P, :].rearrange(
                        "(t p) c -> p t c", t=2
                    ),
                    in_=proj_state["o_sb"],
                )


_NC_CACHE = {}


def build_bass():
    key = (C_SHIFT, tuple(sorted(DVE_JS)))
    if key in _NC_CACHE:
        return _NC_CACHE[key]
    nc = bass.Bass("TRN2")
    with tile.TileContext(nc) as tc:
        with ExitStack() as ctx:
            _emit(nc, tc, ctx)
    _split_multiwaits(nc)
    _NC_CACHE[key] = nc
    return nc


def make_in_maps(x, w_qkv, w_proj):
    x = np.asarray(x, dtype=np.float32)
    w_qkv = np.asarray(w_qkv, dtype=np.float32)
    w_proj = np.asarray(w_proj, dtype=np.float32)
    wq, wk, wv = w_qkv[0:C], w_qkv[C : 2 * C], w_qkv[2 * C : 3 * C]
    in_maps = []
    for c in range(NCORES):
        b, g = divmod(c, 2)
        sl = slice(g * F, (g + 1) * F)
        wslice = np.concatenate([wq[sl], wk[sl], wv[sl]], axis=0)  # [1152, 768]
        wT = np.ascontiguousarray(wslice.T)  # [768, 1152]
        xT = x[b].T  # [768, 2048]
        # [n4][128, co, 512] bf16
        xTn = np.ascontiguousarray(
            xT.reshape(CO, P, 4, 512).transpose(2, 1, 0, 3).astype(ml_dtypes.bfloat16)
        )
        wpT = np.ascontiguousarray(w_proj[:, sl].T)  # [384, 768]
        m = {
            "xTn": xTn,
            "wprojT": np.ascontiguousarray(
                wpT.reshape(FO, P, C).astype(ml_dtypes.bfloat16)
            ),
        }
        for lo, hi in ((0, 128), (384, 512), (768, 1152), (128, 384), (512, 768)):
            m[f"wq{lo}"] = np.ascontiguousarray(wT[:, lo:hi].astype(ml_dtypes.bfloat16))
        in_maps.append(m)
    return in_maps


def gather_output(parts, b_proj):
    outv = np.empty((B, N, C), np.float32)
    for b in range(B):
        outv[b] = parts[2 * b].astype(np.float32) + parts[2 * b + 1].astype(np.float32)
    outv += np.asarray(b_proj, dtype=np.float32)[None, None, :]
    return outv


def kernel(x, w_qkv, w_proj, b_proj, _run_kwargs=None):
    nc = build_bass()
    in_maps = make_in_maps(x, w_qkv, w_proj)
    res = bass_utils.run_bass_kernel_spmd(
        nc, in_maps, core_ids=list(range(NCORES)), **(_run_kwargs or {})
    )
    parts = [r["out3"] for r in res.results]
    outv = gather_output(parts, b_proj)
    if _run_kwargs is not None:
        kernel.last_results = res
    return outv


# revision 7
# speedup vs baseline: 1.0067x; 1.0021x over previous
"""Trainium2 Bass kernel v2 for nn_Attention (B=4, N=2048, C=768, H=12).

Sharding: 8 cores = 4 batches x 2 head-groups (6 heads each). Each core
computes qkv for its 6 heads, attention, and the full projection partial
(all 3 head-pairs summed on-chip); the host adds the 2 group partials + bias.

vs v1:
- exp() emits fp8e4 (e4m3) probabilities directly, with a global logit shift
  C (softmax-invariant) keeping exp in e4m3's dynamic range.
- PV runs as fp8 DoubleRow matmuls (2 j-chunk k-tiles per instruction, 0.5
  cycles/row) with v = v_hi + v_lo error-feedback pair; a ones column in
  v_hi's 65th row gives Z for free. PE time for PV halves vs the fp32r M=65
  scheme.
- The projection contracts all 3 pairs on-chip (bf16) into one bf16 output;
  the host sums 2 partials instead of 6.
- exp is the span-setting engine (~205us ACT); the schedule keeps the ACT
  stream gapless: minimal lead-in (coarse single-DMA weight/x loads, x issued
  from the otherwise-idle ACT queue), PV interleaved with one jj lag so the
  S->exp chain never queues behind PV, and proj lagged one block.
"""

import os
import sys
from contextlib import ExitStack

if "/opt/trn_rl_repo" not in sys.path:
    sys.path.insert(0, "/opt/trn_rl_repo")

import numpy as np
import ml_dtypes

import concourse.bass as bass
import concourse.mybir as mybir
import concourse.tile as tile
from concourse import bass_utils

F32 = mybir.dt.float32
BF16 = mybir.dt.bfloat16
F8 = mybir.dt.float8e4
U8 = mybir.dt.uint8
DRMODE = mybir.MatmulPerfMode.DoubleRow

B, N, C = 4, 2048, 768
NH, D = 12, 64
SCALE = D ** -0.5
HPC = NH // 2          # heads per core
F = HPC * D            # 384 per-core features per projection
P = 128
CO = C // P            # 6 contraction chunks
FO = F // P            # 3 head pairs
NO = N // P            # 16 token chunks
NJJ = NO // 2          # 8 j-chunk pairs (DoubleRow k-tiles)
NCORES = 8
VSTRIDE = 80           # v8hi per-head stride (must be %16==0 for DR lhsT)

C_SHIFT = float(os.environ.get("KERNEL_C_SHIFT", "4.0"))
A_SCH = 8.0 / np.log(2.0)
B_SCH = 55.5
# j-chunk indices (0..15) whose exp runs on the DVE via corrected Schraudolph.
# Default empty: the bit-trick exp fails the 2e-2 gate even at 25% mixing.
_dve_js = os.environ.get("KERNEL_DVE_JS", "")
DVE_JS = set(int(x) for x in _dve_js.split(",") if x != "")

MM_DT = mybir.dt.float32r


def _d(ap):
    return ap.bitcast(MM_DT)


def _r(ap):
    """Cast a producer OUT AP feeding an fp32r matmul (rounding chain)."""
    return ap.bitcast(MM_DT)


def _split_multiwaits(nc):
    """Walrus accepts at most ONE sync-wait per instruction: split extras
    into single-wait NOPs queued just before (FIFO-equivalent)."""
    ctr = 0
    for f in nc.m.functions:
        for blk in f.blocks:
            insts = blk.instructions
            out = []
            changed = False
            for ins in insts:
                si = ins.sync_info
                if si is not None and len(si.on_wait) > 1:
                    changed = True
                    waits = list(si.on_wait)
                    for ww in waits[:-1]:
                        nop = mybir.InstNoOp(name=f"zzsplitw_{ctr}", ins=[], outs=[])
                        ctr += 1
                        nop.engine = ins.engine
                        nop.sync_info = mybir.SyncInfo(on_wait=[ww], on_update=[])
                        out.append(nop)
                    ins.sync_info = mybir.SyncInfo(
                        on_wait=[waits[-1]], on_update=list(si.on_update)
                    )
                out.append(ins)
            if changed:
                blk.instructions = out
    return nc


def _emit(nc, tc, ctx):
    # x n4-major: [n4][128, CO, 512] bf16 so one cheap DMA per 512-token slice
    xTn = nc.dram_tensor("xTn", [4, P, CO, 512], BF16, kind="ExternalInput").ap()
    # five contiguous weight sections [C, w]; loaded with ONE rearranged DMA
    # each into [128, CO, w] (HWDGE descgen is a serial device: fewer DMAs)
    SECS = ((0, P), (F, F + P), (2 * F, 3 * F), (P, F), (F + P, 2 * F))
    wq_secs = {
        lo: nc.dram_tensor(f"wq{lo}", [C, hi - lo], BF16, kind="ExternalInput").ap()
        for lo, hi in SECS
    }
    wprojT = nc.dram_tensor("wprojT", [FO, P, C], BF16, kind="ExternalInput").ap()
    out3 = nc.dram_tensor("out3", [N, C], BF16, kind="ExternalOutput").ap()

    persist = ctx.enter_context(tc.tile_pool(name="persist", bufs=1))

    ones_pe = persist.tile([P, 64], F32, tag="ones_pe")
    warm_in = persist.tile([64, 512], F32, tag="warm_in")
    nc.gpsimd.memset(warm_in, 0.0)
    nc.vector.memset(ones_pe, 1.0)

    # q/k [feature, token] fp32, per (pair, 512-token chunk)
    q_sb = [[persist.tile([P, 512], F32, tag=f"q{fo}_{n4}", name=f"q{fo}_{n4}")
             for n4 in range(4)] for fo in range(FO)]
    k_sb = [[persist.tile([P, 512], F32, tag=f"k{fo}_{n4}", name=f"k{fo}_{n4}")
             for n4 in range(4)] for fo in range(FO)]
    # v fp8 hi/lo per jj: [j-in-chunk, kt, head-strided columns]
    v8hi = [persist.tile([P, 2, HPC * VSTRIDE], F8, tag=f"v8hi{jj}", name=f"v8hi{jj}")
            for jj in range(NJJ)]
    v8lo = [persist.tile([P, 2, HPC * D], F8, tag=f"v8lo{jj}", name=f"v8lo{jj}")
            for jj in range(NJJ)]
    # attention output (normalized) per pair, [feature, token] bf16
    ot_sb = [persist.tile([P, N], BF16, tag=f"ot{pr}", name=f"ot{pr}")
             for pr in range(FO)]
    wp_sb = persist.tile([P, FO, C], BF16, tag="wp")
    # pair-2 odd-head rows of wp copied to partitions 0:64 (tail split-K proj
    # reads the normalized odd half from nt, which lives at partitions 0:64)
    wp2dn = persist.tile([64, C], BF16, tag="wp2dn")

    # ones row at partition 64 for the tail's PE-broadcast of 1/Z (memset
    # first: the PE warm-up dummies depend on it)
    ones_pe0 = None
    # ones columns of v8hi (Z rows)
    for jj in range(NJJ):
        nc.gpsimd.memset(
            v8hi[jj].rearrange("p t (h s) -> p t h s", s=VSTRIDE)[:, :, :, D : D + 1],
            1.0,
        )
    # bias AP for exp's global logit shift (softmax-invariant)
    bias_c = persist.tile([P, 1], F32, tag="biasc")
    nc.vector.memset(bias_c, -C_SHIFT)
    # dummy exp: pull the ACT table load into the DMA lead-in window
    expwarm = persist.tile([P, 4], F32, tag="expwarm")
    nc.vector.memset(expwarm, 0.0)
    nc.scalar.activation(
        out=expwarm, in_=expwarm, func=mybir.ActivationFunctionType.Exp, scale=1.0,
        bias=bias_c,
    )

    with (
        tc.tile_pool(name="wqp", bufs=1) as wqp,
        tc.tile_pool(name="xs", bufs=4) as xs_pool,
        tc.tile_pool(name="p8p", bufs=12) as p8_pool,
        tc.tile_pool(name="rp", bufs=2) as r_pool,
        tc.tile_pool(name="outp", bufs=3) as outp,
        tc.tile_pool(name="rd", bufs=3, space="DRAM") as rd_pool,
        tc.tile_pool(name="ps_st", bufs=2, space="PSUM") as ps_st,
        tc.tile_pool(name="ps_po", bufs=2, space="PSUM") as ps_po,
        tc.tile_pool(name="ps_mm", bufs=2, space="PSUM") as ps_mm,
    ):
        wq_tiles = {}

        def load_wq(lo, hi):
            w = hi - lo
            t = wqp.tile([P, CO, w], BF16, tag=f"wq_{lo}", name=f"wq_{lo}")
            wq_tiles[lo] = t
            nc.sync.dma_start(
                out=t,
                in_=wq_secs[lo].rearrange("(co p) w -> p co w", p=P),
            )

        def wq_slice(foff, co, width=P):
            for lo, hi in SECS:
                if lo <= foff and foff + width <= hi:
                    return wq_tiles[lo][:, co, foff - lo : foff - lo + width]
            raise KeyError(foff)

        # ---- QKV -------------------------------------------------------
        def emit_qk_group(fo, n4, which, xt4):
            """One q or k psum group [128, 512] -> q_sb/k_sb fp32 (bf16 mm)."""
            dst = (q_sb if which == "q" else k_sb)[fo][n4]
            foff = fo * P if which == "q" else F + fo * P
            pq = ps_mm.tile([P, 512], F32, tag="mm", name=f"p{which}{fo}_{n4}")
            for co in range(CO):
                nc.tensor.matmul(
                    pq, wq_slice(foff, co), xt4[:, co, :],
                    start=(co == 0), stop=(co == CO - 1),
                )
            nc.vector.tensor_copy(out=_r(dst), in_=pq)

        def emit_x_dma(n4):
            xt4 = xs_pool.tile([P, CO, 512], BF16, tag="xt4", name=f"xt4_{n4}")
            nc.sync.dma_start(out=xt4, in_=xTn[n4, :, :, :])
            return xt4

        def emit_v_chunk(no, xtv):
            """v for one 128-token chunk -> v8hi/v8lo fp8 (+ones col in hi)."""
            pv = ps_mm.tile([P, F], F32, tag="mm", name=f"pv_{no}")
            for co in range(CO):
                nc.tensor.matmul(
                    pv,
                    xtv[:, co, (no % 4) * P : (no % 4 + 1) * P],
                    wq_slice(2 * F, co, F),
                    start=(co == 0), stop=(co == CO - 1),
                )
            jj, half = divmod(no, 2)
            hi = v8hi[jj].rearrange("p t (h s) -> p t h s", s=VSTRIDE)
            nc.vector.tensor_copy(
                out=hi[:, half, :, 0:D],
                in_=pv.rearrange("p (h d) -> p h d", h=HPC),
            )
            lo = v8lo[jj].rearrange("p t (h d) -> p t h d", d=D)
            nc.vector.tensor_sub(
                out=lo[:, half, :, :],
                in0=pv.rearrange("p (h d) -> p h d", h=HPC),
                in1=hi[:, half, :, 0:D],
            )

        # ---- attention -------------------------------------------------
        def emit_pv(pr, jj, p8, po_a, po_b):
            hA, hB = 2 * pr, 2 * pr + 1
            vh = v8hi[jj]
            vl = v8lo[jj]
            for h, po, icol in ((hA, po_a, 0), (hB, po_b, 512)):
                nc.tensor.matmul(
                    po,
                    vh[:, :, h * VSTRIDE : h * VSTRIDE + 65],
                    p8[:, :, icol : icol + 512],
                    start=(jj == 0), stop=False,
                    perf_mode=DRMODE,
                    skip_group_check=True,
                )
                nc.tensor.matmul(
                    po[0:64, :],
                    vl[:, :, h * D : h * D + 64],
                    p8[:, :, icol : icol + 512],
                    start=False, stop=(jj == NJJ - 1),
                    perf_mode=DRMODE,
                    skip_group_check=True,
                )

        def make_norm(pr, i4, po_a, po_b, pe_bcast=False):
            """Closure emitting the normalize chain for block (pr, i4).

            pe_bcast: broadcast 1/Z across partitions with a K=1 ones-matmul
            into a free PSUM slot instead of the DRAM bounce (tail only --
            saves the DMA round trip and keeps the PE warm for proj).
            """
            i0 = i4 * 512

            def norm_pe():
                rv = r_pool.tile([65, 1024], F32, tag="rv", name=f"rv_{pr}_{i4}")
                with nc.allow_low_precision(reason="1/Z feeds fp32r bcast matmul"):
                    nc.vector.reciprocal(out=_r(rv[64:65, 0:512]), in_=po_a[64:65, :])
                    nc.vector.reciprocal(out=_r(rv[64:65, 512:1024]), in_=po_b[64:65, :])
                rb_ps = ps_st.tile([64, 1024], F32, tag="st", name=f"rbps_{pr}_{i4}")
                for h in range(2):
                    nc.tensor.matmul(
                        rb_ps[:, h * 512 : h * 512 + 512],
                        _d(ones_pe[64:65, :]),
                        _d(rv[64:65, h * 512 : h * 512 + 512]),
                        start=True, stop=True,
                        tile_position=(64, 0),
                    )
                rb_sb = r_pool.tile([64, 2, 512], F32, tag="rb", name=f"rbs_{pr}_{i4}")
                nc.vector.tensor_copy(out=rb_sb, in_=rb_ps.rearrange("p (t n) -> p t n", t=2))
                nc.vector.tensor_mul(
                    out=ot_sb[pr][0:64, i0 : i0 + 512], in0=po_a[0:64, :],
                    in1=rb_sb[:, 0, :],
                )
                nt = r_pool.tile([64, 512], BF16, tag="nt", name=f"nt_{pr}_{i4}")
                nc.vector.tensor_mul(out=nt, in0=po_b[0:64, :], in1=rb_sb[:, 1, :])
                nc.sync.dma_start(out=ot_sb[pr][64:128, i0 : i0 + 512], in_=nt)
                tail_nt["nt"] = nt

            def norm():
                if pe_bcast:
                    return norm_pe()
                rv = r_pool.tile([65, 1024], F32, tag="rv", name=f"rv_{pr}_{i4}")
                nc.vector.reciprocal(out=rv[64:65, 0:512], in_=po_a[64:65, :])
                nc.vector.reciprocal(out=rv[64:65, 512:1024], in_=po_b[64:65, :])
                rdram = rd_pool.tile([1, 1024], F32, tag="rd", name=f"rd_{pr}_{i4}")
                nc.sync.dma_start(out=rdram, in_=rv[64:65, :])
                rb = r_pool.tile([64, 2, 512], F32, tag="rb", name=f"rb_{pr}_{i4}")
                nc.sync.dma_start(out=rb[:, 0, :], in_=rdram[0:1, 0:512].to_broadcast([64, 512]))
                nc.sync.dma_start(out=rb[:, 1, :], in_=rdram[0:1, 512:1024].to_broadcast([64, 512]))
                # multiply straight out of PSUM (DVE reads psum; po slot is
                # not needed again until ~a block later)
                nc.vector.tensor_mul(
                    out=ot_sb[pr][0:64, i0 : i0 + 512], in0=po_a[0:64, :], in1=rb[:, 0, :],
                )
                nt = r_pool.tile([64, 512], BF16, tag="nt", name=f"nt_{pr}_{i4}")
                nc.vector.tensor_mul(out=nt, in0=po_b[0:64, :], in1=rb[:, 1, :])
                nc.sync.dma_start(out=ot_sb[pr][64:128, i0 : i0 + 512], in_=nt)

            return norm

        def emit_attention(pr, interleave=None, cascade=False, carry_in=None):
            """Attention for pair pr over 4 i-blocks of 512.

            cascade=False: PV(jj) inline with a one-jj lag (j = 2jj+3), last
            PV + normalize at block end (pairs 1, 2 -- PE fits in ACT slack).
            cascade=True: block k's 16 PV instructions + its normalize run
            spread inside block k+1's j-loop (pair 0 -- makes room for the v
            projection inside blocks 0-1 without stalling the exp stream).
            The last block's closures are returned as carry for the next
            pair's first block.
            """
            pending = list(carry_in or [])

            def pop_pending():
                if pending:
                    pending.pop(0)()

            for i4 in range(4):
                po_a = ps_po.tile([65, 512], F32, tag="po", name=f"poA_{pr}_{i4}")
                po_b = ps_po.tile([65, 512], F32, tag="po", name=f"poB_{pr}_{i4}")
                p8s = []
                for j in range(NO):
                    kt = k_sb[pr][j // 4]
                    jo = (j % 4) * P
                    qt = q_sb[pr][i4]
                    stm = ps_st.tile([P, 1024], F32, tag="st", name=f"st_{pr}_{i4}_{j}")
                    nc.tensor.matmul(
                        stm[:, 0:512], _d(kt[0:64, jo : jo + P]), _d(qt[0:64, :]),
                        start=True, stop=True,
                    )
                    nc.tensor.matmul(
                        stm[:, 512:1024], _d(kt[64:128, jo : jo + P]), _d(qt[64:128, :]),
                        start=True, stop=True,
                    )
                    if j % 2 == 0:
                        p8 = p8_pool.tile([P, 2, 1024], F8, tag="p8",
                                          name=f"p8_{pr}_{i4}_{j // 2}")
                        p8s.append(p8)
                    p8 = p8s[j // 2]
                    if j in DVE_JS:
                        nc.vector.tensor_scalar(
                            out=p8[:, j % 2, :].bitcast(U8),
                            in0=stm,
                            scalar1=SCALE * A_SCH,
                            scalar2=B_SCH - C_SHIFT * A_SCH,
                            op0=mybir.AluOpType.mult,
                            op1=mybir.AluOpType.add,
                        )
                    else:
                        nc.scalar.activation(
                            out=p8[:, j % 2, :],
                            in_=stm,
                            func=mybir.ActivationFunctionType.Exp,
                            scale=SCALE,
                            bias=bias_c,
                        )
                    if interleave is not None:
                        interleave(i4, j)
                    if cascade:
                        # one pending item per odd j: PV(jj) lands at j=2jj+1,
                        # safely after v(2jj+1)'s write at even j
                        if j % 2 == 1:
                            pop_pending()
                    elif j >= 7 and j % 2 == 1:
                        emit_pv(pr, (j - 7) // 2, p8s[(j - 7) // 2], po_a, po_b)
                if cascade:
                    while pending:
                        pop_pending()
                    for jj in range(NJJ):
                        _jj, _p8 = jj, p8s[jj]
                        pending.append(
                            lambda _jj=_jj, _p8=_p8, _pa=po_a, _pb=po_b: emit_pv(
                                pr, _jj, _p8, _pa, _pb
                            )
                        )
                    pending.append(make_norm(pr, i4, po_a, po_b))
                else:
                    emit_pv(pr, NJJ - 3, p8s[NJJ - 3], po_a, po_b)
                    emit_pv(pr, NJJ - 2, p8s[NJJ - 2], po_a, po_b)
                    emit_pv(pr, NJJ - 1, p8s[NJJ - 1], po_a, po_b)
                    make_norm(pr, i4, po_a, po_b,
                              pe_bcast=(pr == 2 and i4 == 3))()
            return pending

        # ---- projection (all 3 pairs on-chip, bf16) --------------------
        proj_state = {"o_sb": None}
        tail_nt = {}

        def emit_proj(no_range, evac_act=False):
            for no in no_range:
                if no % 2 == 0:
                    proj_state["o_sb"] = outp.tile(
                        [P, 2, C], BF16, tag="o", name=f"o_{no}"
                    )
                o_sb = proj_state["o_sb"]
                for half in range(2):
                    pp = ps_mm.tile([P, 384], F32, tag="mm", name=f"pp_{no}_{half}")
                    for fo in range(FO):
                        nc.tensor.matmul(
                            pp,
                            ot_sb[fo][:, no * P : (no + 1) * P],
                            wp_sb[:, fo, half * 384 : half * 384 + 384],
                            start=(fo == 0), stop=(fo == FO - 1),
                        )
                    dst = o_sb[:, no % 2, half * 384 : half * 384 + 384]
                    if evac_act and half == 0:
                        nc.scalar.copy(out=dst, in_=pp)
                    else:
                        nc.vector.tensor_copy(out=dst, in_=pp)
                if no % 2 == 1:
                    nc.sync.dma_start(
                        out=out3[(no - 1) * P : (no + 1) * P, :].rearrange(
                            "(t p) c -> p t c", t=2
                        ),
                        in_=o_sb,
                    )

        # ---- schedule --------------------------------------------------
        # lead-in (bf16 DMAs pipeline with the k-group chain): wqk, x0, wq0,
        # x1, wv, x2, x3 -- k(0,n4) lands just before the exp stream needs it
        xt4_0 = xs_pool.tile([P, CO, 512], BF16, tag="xt4", name="xt4_0")
        nc.sync.dma_start(out=xt4_0[:, 0, :], in_=xTn[0, :, 0, :])
        nc.sync.dma_start(out=xt4_0[:, 1, :], in_=xTn[0, :, 1, :])
        load_wq(F, F + P)                               # k0 weights
        nc.sync.dma_start(out=xt4_0[:, 2, :], in_=xTn[0, :, 2, :])
        load_wq(0, P)                                   # q0 weights
        for co in range(3, CO):
            nc.sync.dma_start(out=xt4_0[:, co, :], in_=xTn[0, :, co, :])
        xt4s0 = [xt4_0]
        # warm the PE clock while the lead-in DMAs stream (HAM ramp)
        warm_sc = ps_st.tile([64, 1024], F32, tag="st", name="warm_sc")
        for _ in range(10):
            nc.tensor.matmul(
                warm_sc[:, 0:128],
                _d(warm_in[:, 0:64]),
                _d(warm_in[:, 0:128]),
                start=True, stop=True,
            )
        xt4s0.append(emit_x_dma(1))
        load_wq(2 * F, 3 * F)                           # v weights
        xt4s0.append(emit_x_dma(2))
        xt4s0.append(emit_x_dma(3))
        # fused k(0,0)+q(0,0): co-matmuls pipelined against x0 chunk arrivals
        pk0 = ps_mm.tile([P, 512], F32, tag="mm", name="pk00")
        pq0 = ps_mm.tile([P, 512], F32, tag="mm", name="pq00")
        for co in range(CO):
            nc.tensor.matmul(
                pk0, wq_slice(F, co), xt4_0[:, co, :],
                start=(co == 0), stop=(co == CO - 1),
            )
            nc.tensor.matmul(
                pq0, wq_slice(0, co), xt4_0[:, co, :],
                start=(co == 0), stop=(co == CO - 1),
            )
        nc.vector.tensor_copy(out=_r(k_sb[0][0]), in_=pk0)
        nc.vector.tensor_copy(out=_r(q_sb[0][0]), in_=pq0)
        nc.sync.dma_start(
            out=wp_sb, in_=wprojT.rearrange("fo p c -> p fo c"),
        )
        nc.sync.dma_start(out=wp2dn, in_=wprojT[2, 64:128, :])
        load_wq(P, F)                                   # q1/q2
        load_wq(F + P, 2 * F)                           # k1/k2

        # pair 0 (cascade PV): v chunks spread over blocks 0-1, q(0,i4+1)
        # at j==14, pair-1 qkv groups in blocks 2-3
        x_cache = {}
        qk1_slots = [(2, 2), (2, 6), (2, 10), (3, 2), (3, 4), (3, 8), (3, 10), (3, 12)]
        qk2_slots = [(0, 4), (0, 10), (1, 4), (1, 10), (2, 4), (2, 10), (3, 4), (3, 10)]
        qkn4 = [0, 1, 2, 3, 0, 1, 2, 3]

        def emit_qk_pair_group(pr, idx):
            n4 = qkn4[idx]
            if (pr, n4) not in x_cache:
                if pr == 2:
                    x_cache[(pr, n4)] = x_cache[(1, n4)]
                else:
                    x_cache[(pr, n4)] = emit_x_dma(n4)
            xt4 = x_cache[(pr, n4)]
            emit_qk_group(pr, n4, "q" if idx < 4 else "k", xt4)

        def inter0(i4, j):
            # k(0,1..3) pipelined against the x1..x3 DMA arrivals
            if i4 == 0 and j in (1, 5, 9):
                emit_qk_group(0, 1 + (j - 1) // 4, "k", xt4s0[1 + (j - 1) // 4])
            # v(no): 10 chunks late in block 0 (after wv lands), 6 in block 1
            if i4 == 0 and 6 <= j:
                emit_v_chunk(j - 6, xt4s0[(j - 6) // 4])
            elif i4 == 1 and j % 2 == 0 and 2 <= j <= 12:
                emit_v_chunk(10 + (j - 2) // 2, xt4s0[(10 + (j - 2) // 2) // 4])
            if i4 == 0 and j == 14:
                emit_qk_group(0, 1, "q", xt4s0[1])
            elif i4 in (1, 2) and j == 9:
                emit_qk_group(0, i4 + 1, "q", xt4s0[i4 + 1])
            if (i4, j) in qk1_slots:
                emit_qk_pair_group(1, qk1_slots.index((i4, j)))

        carry = emit_attention(0, interleave=inter0, cascade=True)

        def inter1(i4, j):
            if (i4, j) in qk2_slots:
                emit_qk_pair_group(2, qk2_slots.index((i4, j)))
            # drain pair-0 block-3's carried PVs + norm in block 0
            if i4 == 0 and j % 2 == 1 and carry:
                carry.pop(0)()
                if j == 13:
                    while carry:
                        carry.pop(0)()

        emit_attention(1, interleave=inter1)

        # attention 2: proj for block i4-1's chunks lands inside block i4
        def inter2(i4, j):
            if i4 >= 1 and j in (6, 9, 12, 15):
                no = 4 * (i4 - 1) + (6, 9, 12, 15).index(j)
                emit_proj([no])

        emit_attention(2, interleave=inter2)
        for _ in range(10):
            nc.tensor.matmul(
                warm_sc[:, 0:64], _d(ones_pe[0:64, :]), _d(ones_pe[0:64, :]),
                start=True, stop=True,
            )
        # tail proj: split pair-2's contraction so the odd-head half reads nt
        # (SBUF partitions 0:64) instead of waiting for the ot DMA-up
        nt3 = tail_nt["nt"]
        for no in range(12, 16):
            if no % 2 == 0:
                proj_state["o_sb"] = outp.tile([P, 2, C], BF16, tag="o", name=f"o_{no}")
            o_sb = proj_state["o_sb"]
            for half in range(2):
                pp = ps_mm.tile([P, 384], F32, tag="mm", name=f"pp_{no}_{half}")
                hc = slice(half * 384, half * 384 + 384)
                for fo in range(2):
                    nc.tensor.matmul(
                        pp, ot_sb[fo][:, no * P : (no + 1) * P], wp_sb[:, fo, hc],
                        start=(fo == 0), stop=False, skip_group_check=True,
                    )
                nc.tensor.matmul(
                    pp, ot_sb[2][0:64, no * P : (no + 1) * P], wp_sb[0:64, 2, hc],
                    start=False, stop=False, skip_group_check=True,
                )
                nc.tensor.matmul(
                    pp, nt3[:, (no % 4) * P : (no % 4 + 1) * P], wp2dn[:, hc],
                    start=False, stop=True, skip_group_check=True,
                )
                dst = o_sb[:, no % 2, hc]
                if half == 0:
                    nc.scalar.copy(out=dst, in_=pp)
                else:
                    nc.vector.tensor_copy(out=dst, in_=pp)
            if no >= 14:
                nc.sync.dma_start(
                    out=out3[no * P : (no + 1) * P, :],
                    in_=proj_state["o_sb"][:, no % 2, :],
                )
            elif no % 2 == 1:
                nc.sync.dma_start(
                    out=out3[(no - 1) * P : (no + 1) * P, :].rearrange(
                        "(t p) c -> p t c", t=2
                    ),
                    in_=proj_state["o_sb"],
                )


_NC_CACHE = {}


def build_bass():
    key = (C_SHIFT, tuple(sorted(DVE_JS)))
    if key in _NC_CACHE:
        return _NC_CACHE[key]
    nc = bass.Bass("TRN2")
    with tile.TileContext(nc) as tc:
        with ExitStack() as ctx:
            _emit(nc, tc, ctx)
    _split_multiwaits(nc)
    _NC_CACHE[key] = nc
    return nc


def make_in_maps(x, w_qkv, w_proj):
    x = np.asarray(x, dtype=np.float32)
    w_qkv = np.asarray(w_qkv, dtype=np.float32)
    w_proj = np.asarray(w_proj, dtype=np.float32)
    wq, wk, wv = w_qkv[0:C], w_qkv[C : 2 * C], w_qkv[2 * C : 3 * C]
    in_maps = []
    for c in range(NCORES):
        b, g = divmod(c, 2)
        sl = slice(g * F, (g + 1) * F)
        wslice = np.concatenate([wq[sl], wk[sl], wv[sl]], axis=0)  # [1152, 768]
        wT = np.ascontiguousarray(wslice.T)  # [768, 1152]
        xT = x[b].T  # [768, 2048]
        # [n4][128, co, 512] bf16
        xTn = np.ascontiguousarray(
            xT.reshape(CO, P, 4, 512).transpose(2, 1, 0, 3).astype(ml_dtypes.bfloat16)
        )
        wpT = np.ascontiguousarray(w_proj[:, sl].T)  # [384, 768]
        m = {
            "xTn": xTn,
            "wprojT": np.ascontiguousarray(
                wpT.reshape(FO, P, C).astype(ml_dtypes.bfloat16)
            ),
        }
        for lo, hi in ((0, 128), (384, 512), (768, 1152), (128, 384), (512, 768)):
            m[f"wq{lo}"] = np.ascontiguousarray(wT[:, lo:hi].astype(ml_dtypes.bfloat16))
        in_maps.append(m)
    return in_maps


def gather_output(parts, b_proj):
    outv = np.empty((B, N, C), np.float32)
    for b in range(B):
        outv[b] = parts[2 * b].astype(np.float32) + parts[2 * b + 1].astype(np.float32)
    outv += np.asarray(b_proj, dtype=np.float32)[None, None, :]
    return outv


def kernel(x, w_qkv, w_proj, b_proj, _run_kwargs=None):
    nc = build_bass()
    in_maps = make_in_maps(x, w_qkv, w_proj)
    res = bass_utils.run_bass_kernel_spmd(
        nc, in_maps, core_ids=list(range(NCORES)), **(_run_kwargs or {})
    )
    parts = [r["out3"] for r in res.results]
    outv = gather_output(parts, b_proj)
    if _run_kwargs is not None:
        kernel.last_results = res
    return outv
